# revision 30
# baseline (speedup 1.0000x reference)
"""Trainium2 Bass kernel for nn_DepthwiseXCorr (SiamRPN-style depthwise
cross-correlation head), data-parallel over 8 NeuronCores.

Network (per sample):
  k = relu(bn(conv3x3(kernel)))      [256,7,7]   -> [256,5,5]
  s = relu(bn(conv3x3(search)))      [256,31,31] -> [256,29,29]
  feat = depthwise_xcorr(s, k)                   -> [256,25,25]
  h = relu(bn(conv1x1(feat)))                    -> [256,25,25]
  out = conv1x1(h) + b                           -> [256,25,25]

Mapping (v2, fp8-DoubleRow design):
  - batch 128 sharded 16 samples/core across 8 cores (SPMD, no collectives)
  - BN folded into conv weights/biases on host
  - conv_search in fp8e4 DoubleRow ("cs3t"): x and w split hi+lo on host,
    three 256-deep 0.5-cyc/col passes (wh*xh + wh*xl + wl*xh) ~ fp16-exact
    at 0.75x the fp16 cycle count; conv_kernel + heads stay fp16
  - one conv tap drops its w_lo correction (CS_M=1): ~0.7e-2 extra error
    for 2 fewer DoubleRow passes per sample
  - depthwise xcorr split across four lanes per 25-tap channel-chunk:
      * XC_PE16 taps: fp16 diagonal-weight matmuls on the PE (as baseline)
      * XC_ACT8/XC_POOL8 taps: ACT/Pool copy-with-scale products written
        as fp8 into pair tiles, pairs folded into PSUM by a 65-ns
        constant-[2I,2I] fp8 DoubleRow matmul on the PE (pairs mix one ACT
        and one Pool product so folds never wait two serial ACT ops)
      * remaining taps: DVE tensor_scalar products (fp16 4x) + pair-add
        chain (fp16 2x)
    the fp8 product rounding (~3.6% rms per tap) is the dominant added
    noise; lane counts keep total rel err ~1.5e-2 vs the 2e-2 gate
  - software pipeline: PE fold phase for sample s-1 and heads for s-2 are
    emitted under conv_search(s), so the PE never head-of-line blocks on
    vector-engine product streams; out-DMAs issue from the idle SP queue
  - PSUM banks: 2 conv + 2 head + 4 xcorr partials
"""
import os
import numpy as np

import bass_rust
import concourse.bass as bass
import concourse.mybir as mybir
import concourse.tile as tile
from concourse.bass_utils import run_bass_kernel_spmd

dt = mybir.dt
F32, F16, F8 = dt.float32, dt.float16, dt.float8e4
AF = mybir.ActivationFunctionType
ALU = mybir.AluOpType
DR = mybir.MatmulPerfMode.DoubleRow

N_CORES = 8
B, CIN, HID, OC = 128, 256, 256, 256
SPC = B // N_CORES  # samples per core (16)
EPS = 1e-5

KW = 8                          # kernel input row padded 7 -> 8
SW = 32                         # search input row padded 31 -> 32
SFW = 29                        # conv_search output row
FW = 25                         # xcorr/head output row
KCOLS = SPC * 25                # conv_kernel psum free size (all samples)
S_X = 32.0                      # host fp8 scale for search input (hi and lo)

_SFY0 = int(os.environ.get("SFY0", "15"))
SFY = [(0, _SFY0), (_SFY0, 29 - _SFY0)]  # conv_search output row halves
_HN1 = int(os.environ.get("HN1", "313"))
HN = [(0, _HN1), (_HN1, 625 - _HN1)]  # head matmul N splits of 625
_XCH0 = int(os.environ.get("XCH0", "13"))
XCH = [(0, _XCH0), (_XCH0, 25 - _XCH0)]  # xcorr row halves for PE psum

# xcorr lane counts per channel-chunk (cc0, cc1); rest of 25 goes to DVE
XC_PE16 = int(os.environ.get("XC_PE16", "4"))     # fp16 diag taps / cc
XC_KS = int(os.environ.get("XC_KS", "0"))         # k-split fp8 diag taps / cc
XC_ACT8 = int(os.environ.get("XC_ACT8", "6"))     # ACT fp8-product taps / cc
XC_POOL8 = int(os.environ.get("XC_POOL8", "4"))   # Pool fp8-product taps / cc
XC_POOL16 = int(os.environ.get("XC_POOL16", "0"))  # DVE-lane products on Pool / cc
XC_DIAG_POOL = int(os.environ.get("XC_DIAG_POOL", "0"))  # build fp16 diags on Pool
XC_PE16_LAST = int(os.environ.get("XC_PE16_LAST", "4"))  # tail: extra PE taps
XC_ACT_LAST = int(os.environ.get("XC_ACT_LAST", "4"))  # tail: ACT product cap
XC_TAIL = int(os.environ.get("XC_TAIL", "1"))     # samples treated as tail
OUT_DMA_ENG = os.environ.get("OUT_DMA_ENG", "sync")  # gpsimd|scalar|sync
CK_FIRST = int(os.environ.get("CK_FIRST", "1"))   # conv_kernel before search0
XC_WARM = int(os.environ.get("XC_WARM", "8"))     # PE warm-up matmuls
PSA = int(os.environ.get("PSA", "2"))   # conv psum bufs
PSB = int(os.environ.get("PSB", "2"))   # heads psum bufs
PSX = int(os.environ.get("PSX", "4"))   # xcorr psum bufs
SFB = int(os.environ.get("SFB", "2"))   # search-feature bufs
EV_DVE = int(os.environ.get("EV_DVE", "0"))  # conv_search evacs on DVE
CS_M = int(os.environ.get("CS_M", "1"))      # taps skipping the w_lo term
XS_M = int(os.environ.get("XS_M", "0"))      # taps skipping the x_lo term
XC_CHAIN_POOL = int(os.environ.get("XC_CHAIN_POOL", "0"))  # chain TTs on Pool/cc
XC_LAST_MODE = int(os.environ.get("XC_LAST_MODE", "0"))  # 1: last sample PE+DVE only
XC_FILL = int(os.environ.get("XC_FILL", "0"))  # 1: tight (no-lag) sample 0


def _split_multi_waits(nc):
    """This walrus build accepts at most ONE sync wait per instruction;
    Tile's wait assignment can attach several. Move extras onto prepended
    same-engine NoOps (engine streams are in-order, semantics identical)."""
    n = 0
    for fn in nc.m.functions:
        for bb in fn.blocks:
            changed = False
            out = []
            for inst in bb.instructions:
                si = inst.sync_info
                waits = list(si.on_wait) if si is not None and si.on_wait else []
                if len(waits) > 1:
                    for w in waits[:-1]:
                        no = bass_rust.InstNoOp(
                            name=nc.get_next_instruction_name(), ins=[], outs=[])
                        no.engine = inst.engine
                        no.sync_info = bass_rust.SyncInfo(on_wait=[w], on_update=[])
                        out.append(no)
                    inst.sync_info = bass_rust.SyncInfo(
                        on_wait=[waits[-1]],
                        on_update=list(si.on_update) if si.on_update else [])
                    changed = True
                    n += 1
                out.append(inst)
            if changed:
                bb.instructions = out
    return n


def _shifted(ap, extra_offset, free_dims):
    """Rebuild an SBUF tile AP with a free-dim window: keep partition dim,
    replace free dims, add an element offset."""
    return bass.AP(ap.tensor, ap.offset + extra_offset,
                   [list(ap.ap[0])] + [list(d) for d in free_dims])


def _build(n_samples=SPC):
    nc = bass.Bass(trn_type="TRN2", target_bir_lowering=False, debug=False)

    xk = nc.dram_tensor("xk", [2, 128, SPC * 7 * KW], F16, kind="ExternalInput")
    # search input hi/lo fp8: [s][128][ci_chunk*992 + y*32 + x]
    xsh = nc.dram_tensor("xsh", [SPC, 128, 2 * 31 * SW], F8, kind="ExternalInput")
    xsl = nc.dram_tensor("xsl", [SPC, 128, 2 * 31 * SW], F8, kind="ExternalInput")
    wkt = nc.dram_tensor("wkt", [2, 128, 9 * 256], F16, kind="ExternalInput")
    # conv_search weights hi/lo fp8: [128ci_p][ci_chunk*2304 + tap*256 + co]
    wsh = nc.dram_tensor("wsh", [128, 2 * 9 * 256], F8, kind="ExternalInput")
    wsl = nc.dram_tensor("wsl", [128, 2 * 9 * 256], F8, kind="ExternalInput")
    wh1t = nc.dram_tensor("wh1t", [2, 128, 256], F16, kind="ExternalInput")
    wh2t = nc.dram_tensor("wh2t", [2, 128, 256], F16, kind="ExternalInput")
    # bias cols: 0=bk 1=bs 2=bh1 3=bh2 4=0.5*bk 5=conv_search evac scale
    bias = nc.dram_tensor("bias", [2, 128, 6], F32, kind="ExternalInput")
    out = nc.dram_tensor("out", [2, SPC, 128, 625], F32, kind="ExternalOutput")

    with tile.TileContext(nc) as tc:
        with tc.tile_pool(name="w", bufs=1) as wp, \
             tc.tile_pool(name="xsp", bufs=int(os.environ.get("XSB", "3"))) as xsp, \
             tc.tile_pool(name="sfp", bufs=SFB, space="SBUF") as sfp, \
             tc.tile_pool(name="prp", bufs=3) as prp, \
             tc.tile_pool(name="p8p", bufs=5) as p8p, \
             tc.tile_pool(name="accp", bufs=int(os.environ.get("ACCB", "2"))) as accp, \
             tc.tile_pool(name="featp", bufs=int(os.environ.get("FTB", "2"))) as featp, \
             tc.tile_pool(name="dgp", bufs=2) as dgp, \
             tc.tile_pool(name="sf8p", bufs=2) as sf8p, \
             tc.tile_pool(name="dg2p", bufs=2) as dg2p, \
             tc.tile_pool(name="hp", bufs=int(os.environ.get("HPB", "2"))) as hp, \
             tc.tile_pool(name="obp", bufs=int(os.environ.get("OBB", "2"))) as obp, \
             tc.tile_pool(name="psA", bufs=PSA, space="PSUM") as psA, \
             tc.tile_pool(name="psB", bufs=PSB, space="PSUM") as psB, \
             tc.tile_pool(name="psX", bufs=PSX, space="PSUM") as psX:

            # ---- resident weights / biases / kernel-branch input ----
            wk_sb, w1_sb, w2_sb, bias_sb, xk_sb = [], [], [], [], []
            for c in range(2):
                t = wp.tile([128, 9 * 256], F16, tag=f"wk{c}", name=f"wk{c}")
                nc.sync.dma_start(out=t[:], in_=wkt.ap()[c])
                wk_sb.append(t)
                t = wp.tile([128, SPC * 7 * KW], F16, tag=f"xk{c}", name=f"xk{c}")
                nc.sync.dma_start(out=t[:], in_=xk.ap()[c])
                xk_sb.append(t)
                t = wp.tile([128, 6], F32, tag=f"bias{c}", name=f"bias{c}")
                nc.sync.dma_start(out=t[:], in_=bias.ap()[c])
                bias_sb.append(t)
            XS0_EARLY = int(os.environ.get("XS0_EARLY", "0"))
            xw0_early = None
            if CK_FIRST and XS0_EARLY:
                th0 = xsp.tile([128, 2 * 31 * SW], F8, tag="xsh", name="t_xsh")
                nc.sync.dma_start(out=th0[:], in_=xsh.ap()[0])
                tl0 = xsp.tile([128, 2 * 31 * SW], F8, tag="xsl", name="t_xsl")
                nc.sync.dma_start(out=tl0[:], in_=xsl.ap()[0])
                xw0_early = (th0, tl0)
            ws_hi = wp.tile([128, 2 * 9 * 256], F8, tag="ws_hi", name="ws_hi")
            nc.sync.dma_start(out=ws_hi[:], in_=wsh.ap())
            ws_lo = wp.tile([128, 2 * 9 * 256], F8, tag="ws_lo", name="ws_lo")
            nc.sync.dma_start(out=ws_lo[:], in_=wsl.ap())

            def load_head_weights():
                for c in range(2):
                    t = wp.tile([128, 256], F16, tag=f"w1{c}", name=f"w1{c}")
                    nc.sync.dma_start(out=t[:], in_=wh1t.ap()[c])
                    w1_sb.append(t)
                    t = wp.tile([128, 256], F16, tag=f"w2{c}", name=f"w2{c}")
                    nc.sync.dma_start(out=t[:], in_=wh2t.ap()[c])
                    w2_sb.append(t)

            from concourse.masks import make_identity
            iden = wp.tile([128, 128], F32, tag="iden", name="iden")
            make_identity(nc, iden[:])
            iden16 = wp.tile([128, 128], F16, tag="iden16", name="iden16")
            nc.vector.tensor_copy(out=iden16[:], in_=iden[:])
            # constant [2I, 2I] fp8 pair-fold weights
            iden8x2 = wp.tile([128, 256], F8, tag="iden8x2", name="iden8x2")
            nc.vector.tensor_scalar_mul(iden8x2[:, :128], iden[:], 2.0)
            nc.vector.tensor_scalar_mul(iden8x2[:, 128:], iden[:], 2.0)

            def dr_lhsT(tile_ap, offset, stride):
                return bass.AP(tile_ap.tensor, tile_ap.offset + offset,
                               [list(tile_ap.ap[0]), [stride, 2], [1, 128]])

            # ---- conv_kernel: all samples batched in the free dim ----
            kf_sb = []    # fp32 k columns per cc: [128, s*25 + t]
            kf8_sb = []   # fp32 0.5*k columns per cc (fp8 product lanes)
            kl_sb = []    # fp32 k-residual columns per cc (k-split diags)

            def emit_conv_kernel():
                for co in range(2):
                    ps = psA.tile([128, KCOLS], F32, tag="ps", name="ck_ps")
                    n_mm = 0
                    for tap in range(9):
                        dy, dx = divmod(tap, 3)
                        for ci in range(2):
                            rhs = _shifted(xk_sb[ci][:], dy * KW + dx,
                                           [[7 * KW, n_samples], [KW, 5], [1, 5]])
                            lhs = wk_sb[ci][:, tap * 256 + co * 128:tap * 256 + co * 128 + 128]
                            n_cols = n_samples * 25
                            nc.tensor.matmul(out=ps[:, :n_cols], lhsT=lhs, rhs=rhs,
                                             start=(n_mm == 0), stop=(n_mm == 17))
                            n_mm += 1
                    kf = wp.tile([128, KCOLS], F32, tag=f"kf{co}", name=f"kf{co}")
                    nc.scalar.activation(out=kf[:], in_=ps[:], func=AF.Relu,
                                         bias=bias_sb[co][:, 0:1], scale=1.0)
                    kf_sb.append(kf)
                    kf8 = wp.tile([128, KCOLS], F32, tag=f"kf8{co}", name=f"kf8{co}")
                    nc.scalar.activation(out=kf8[:], in_=ps[:], func=AF.Relu,
                                         bias=bias_sb[co][:, 4:5], scale=0.5)
                    kf8_sb.append(kf8)
                    if XC_KS > 0:
                        kf8c = wp.tile([128, KCOLS], F8, tag=f"kf8c{co}",
                                       name=f"kf8c{co}")
                        nc.vector.tensor_copy(out=kf8c[:], in_=kf[:])
                        klc = wp.tile([128, KCOLS], F32, tag=f"klc{co}",
                                      name=f"klc{co}")
                        nc.vector.tensor_tensor(out=klc[:], in0=kf[:],
                                                in1=kf8c[:], op=ALU.subtract)
                        kl_sb.append(klc)

            def emit_conv_search_x(s):
                th = xsp.tile([128, 2 * 31 * SW], F8, tag="xsh", name="t_xsh")
                nc.sync.dma_start(out=th[:], in_=xsh.ap()[s])
                tl = xsp.tile([128, 2 * 31 * SW], F8, tag="xsl", name="t_xsl")
                nc.sync.dma_start(out=tl[:], in_=xsl.ap()[s])
                return (th, tl)

            def emit_conv_search(s, xw=None):
                if xw is None:
                    xw = emit_conv_search_x(s)
                th, tl = xw

                def xwin(t, ys, nr, dy, dx):
                    return bass.AP(t[:].tensor, t[:].offset + (ys + dy) * SW + dx,
                                   [list(t[:].ap[0]), [31 * SW, 2],
                                    [SW, nr], [1, SFW]])

                sf = []
                for co in range(2):
                    sft = sfp.tile([128, 29 * SFW], F16, tag=f"sf{co}",
                                   name=f"sf{co}")
                    for (ys, nr) in SFY:
                        ps = psA.tile([128, SFY[0][1] * SFW], F32, tag="ps",
                                      name="cs_ps")
                        n_tot = 27 - CS_M - XS_M
                        n_mm = 0
                        for tap in range(9):
                            dy, dx = divmod(tap, 3)
                            passes = [(ws_hi, (th, tl) if tap >= XS_M else (th,))]
                            if tap < 9 - CS_M:
                                passes.append((ws_lo, (th,)))
                            for wtile, xts in passes:
                                lhs = dr_lhsT(wtile[:], tap * 256 + co * 128,
                                              9 * 256)
                                for xt in xts:
                                    nc.tensor.matmul(
                                        out=ps[:, :nr * SFW], lhsT=lhs,
                                        rhs=xwin(xt, ys, nr, dy, dx),
                                        start=(n_mm == 0), stop=(n_mm == n_tot - 1),
                                        perf_mode=DR)
                                    n_mm += 1
                        nc.scalar.activation(
                            out=sft[:, ys * SFW:(ys + nr) * SFW],
                            in_=ps[:, :nr * SFW], func=AF.Relu,
                            bias=bias_sb[co][:, 1:2],
                            scale=bias_sb[co][:, 5:6])
                    sf.append(sft)
                return sf

            def win_of(sf, cc, t, rows=25, row0=0):
                ty, tx = divmod(t, 5)
                return _shifted(sf[cc][:], (row0 + ty) * SFW + tx,
                                [[SFW, rows], [1, FW]])

            def kcol(arr, cc, s, t):
                return arr[cc][:, s * 25 + t:s * 25 + t + 1]

            def ptile():
                t = prp.tile([128, 1250], F16, tag=f"pr{ptile.i % 6}",
                             name=f"pr{ptile.i % 6}")
                ptile.i += 1
                return t
            ptile.i = 0

            def p8tile():
                t = p8p.tile([128, 1250], F8, tag=f"p8_{p8tile.i % 6}",
                             name=f"p8_{p8tile.i % 6}")
                p8tile.i += 1
                return t
            p8tile.i = 0

            def d2tile():
                t = dg2p.tile([128, 256], F8, tag=f"d2_{d2tile.i % 10}",
                              name=f"d2_{d2tile.i % 10}")
                d2tile.i += 1
                return t
            d2tile.i = 0

            def dtile():
                t = dgp.tile([128, 128], F16, tag=f"dg{dtile.i % 12}",
                             name=f"dg{dtile.i % 12}")
                dtile.i += 1
                return t
            dtile.i = 0

            def emit_xcorr_products(s, sf, last=False):
                """Phase 1: lane assignment, fp8/fp16 products, diag builds,
                DVE chain. Returns state for the PE + assembly phases."""
                state = []
                for cc in range(2):
                    n_pe = XC_PE16
                    n_act, n_pool = XC_ACT8, XC_POOL8
                    if last >= 2:
                        # very last sample: nothing left to overlap ACT/Pool
                        # products with -- keep the drain on PE + DVE
                        n_pe = XC_PE16_LAST
                        n_act = n_pool = 0
                    elif last:
                        n_pe = XC_PE16_LAST
                        tot8 = min(n_act + n_pool, 25 - n_pe)
                        n_act = min(n_act, tot8, XC_ACT_LAST)
                        n_pool = tot8 - n_act
                    n8 = n_act + n_pool
                    if (n8 % 2) == 1:
                        n8 -= 1
                        if n_pool > 0:
                            n_pool -= 1
                        else:
                            n_act -= 1
                    n_ks = 0 if last else XC_KS
                    n_pe = min(n_pe, 25 - n8 - n_ks)
                    n_dve = 25 - n8 - n_ks - n_pe
                    dve_taps = list(range(n_dve))
                    f8_taps = list(range(n_dve, n_dve + n8))
                    ks_taps = list(range(n_dve + n8, n_dve + n8 + n_ks))
                    pe_taps = list(range(n_dve + n8 + n_ks, 25))

                    use_psum = bool(pe_taps or f8_taps or ks_taps)
                    n_fold = n8 // 2

                    # fp8 products into pair tiles; pairs mix (ACT, Pool) so a
                    # fold never waits two sequential ACT ops
                    pr8s = []
                    act_left, pool_left = n_act, n_pool
                    for j in range(n_fold):
                        ta, tb = f8_taps[2 * j], f8_taps[2 * j + 1]
                        pr8 = p8tile()
                        for slot, t in ((0, ta), (1, tb)):
                            dst = pr8[:, slot * 625:(slot + 1) * 625]
                            use_act = (act_left > 0 and (slot == 0 or pool_left == 0))
                            if use_act:
                                act_left -= 1
                                nc.scalar.activation(
                                    out=dst, in_=win_of(sf, cc, t),
                                    func=AF.Copy, scale=kcol(kf8_sb, cc, s, t))
                            else:
                                pool_left -= 1
                                nc.gpsimd.tensor_scalar(
                                    out=dst, in0=win_of(sf, cc, t),
                                    scalar1=kcol(kf8_sb, cc, s, t),
                                    scalar2=None, op0=ALU.mult, op1=ALU.bypass)
                        pr8s.append(pr8)

                    # fp16 diag builds for the PE taps (Pool has slack and
                    # the diags are consumed a lagged sample later)
                    dgs = []
                    for t in pe_taps:
                        dg = dtile()
                        if XC_DIAG_POOL:
                            nc.gpsimd.tensor_scalar(
                                out=dg[:], in0=iden16[:],
                                scalar1=kcol(kf_sb, cc, s, t), scalar2=None,
                                op0=ALU.mult, op1=ALU.bypass)
                        else:
                            nc.vector.tensor_scalar_mul(
                                dg[:], iden16[:], kcol(kf_sb, cc, s, t))
                        dgs.append(dg)

                    # k-split fp8 diag pairs + fp8 search tile (Pool-built)
                    sf8t = None
                    dg2s = []
                    if ks_taps:
                        sf8t = sf8p.tile([128, 29 * SFW], F8, tag=f"s8{cc}",
                                         name=f"s8{cc}")
                        nc.gpsimd.tensor_copy(out=sf8t[:], in_=sf[cc][:])
                        for t in ks_taps:
                            d2 = d2tile()
                            nc.gpsimd.tensor_scalar(
                                out=d2[:, :128], in0=iden16[:],
                                scalar1=kcol(kf_sb, cc, s, t), scalar2=None,
                                op0=ALU.mult, op1=ALU.bypass)
                            nc.gpsimd.tensor_scalar(
                                out=d2[:, 128:], in0=iden16[:],
                                scalar1=kcol(kl_sb, cc, s, t), scalar2=None,
                                op0=ALU.mult, op1=ALU.bypass)
                            dg2s.append(d2)

                    # DVE lane: TS products into fp16 pair tiles + TT chain;
                    # the first XC_POOL16 products go on the Pool engine
                    pairs = []
                    single = None
                    kk = 0
                    n_p16 = XC_POOL16

                    def prod16(dst, t):
                        nonlocal n_p16
                        if n_p16 > 0:
                            n_p16 -= 1
                            nc.gpsimd.tensor_scalar(
                                out=dst, in0=win_of(sf, cc, t),
                                scalar1=kcol(kf_sb, cc, s, t), scalar2=None,
                                op0=ALU.mult, op1=ALU.bypass)
                        else:
                            nc.vector.tensor_scalar_mul(
                                dst, win_of(sf, cc, t), kcol(kf_sb, cc, s, t))

                    while kk < len(dve_taps):
                        pr = ptile()
                        prod16(pr[:, :625], dve_taps[kk])
                        if kk + 1 < len(dve_taps):
                            prod16(pr[:, 625:], dve_taps[kk + 1])
                            pairs.append(pr)
                            kk += 2
                        else:
                            single = pr
                            kk += 1

                    a0 = accp.tile([128, 1250], F16, tag=f"ac{cc}a",
                                   name=f"ac{cc}a")
                    a1 = accp.tile([128, 1250], F16, tag=f"ac{cc}b",
                                   name=f"ac{cc}b")
                    accs, nxt = [a0, a1], 0
                    cur2 = None
                    n_chain_pool = XC_CHAIN_POOL
                    for pr in pairs:
                        if cur2 is None:
                            cur2 = pr[:]
                            continue
                        d = accs[nxt][:]
                        if n_chain_pool > 0:
                            n_chain_pool -= 1
                            nc.gpsimd.tensor_tensor(out=d, in0=cur2, in1=pr[:],
                                                    op=ALU.add)
                        else:
                            nc.vector.tensor_tensor(out=d, in0=cur2, in1=pr[:],
                                                    op=ALU.add)
                        cur2, nxt = d, 1 - nxt
                    # fold chain halves into [128,625]
                    chain = None
                    if cur2 is not None:
                        ch = accp.tile([128, 625], F16, tag=f"ch{cc}",
                                       name=f"ch{cc}")
                        h0 = bass.AP(cur2.tensor, cur2.offset,
                                     [list(cur2.ap[0]), [1, 625]])
                        h1 = bass.AP(cur2.tensor, cur2.offset + 625,
                                     [list(cur2.ap[0]), [1, 625]])
                        if single is not None:
                            # h0+h1 then +single via two TTs
                            nc.vector.tensor_tensor(out=ch[:], in0=h0, in1=h1,
                                                    op=ALU.add)
                            ch2 = accp.tile([128, 625], F16, tag=f"ch2{cc}",
                                            name=f"ch2{cc}")
                            nc.vector.tensor_tensor(out=ch2[:], in0=ch[:],
                                                    in1=single[:, :625],
                                                    op=ALU.add)
                            chain = ch2[:]
                        else:
                            nc.vector.tensor_tensor(out=ch[:], in0=h0, in1=h1,
                                                    op=ALU.add)
                            chain = ch[:]
                    elif single is not None:
                        chain = single[:, :625]

                    state.append(dict(cc=cc, use_psum=use_psum, chain=chain,
                                      pe_taps=pe_taps, dgs=dgs, pr8s=pr8s,
                                      ks_taps=ks_taps, dg2s=dg2s, sf8t=sf8t,
                                      n_fold=n_fold, sf=sf, s=s))
                return state

            def emit_xcorr_pe(state):
                """Phase 2: PE diag matmuls + fp8 pair folds into PSUM."""
                for st in state:
                    sf, s = st["sf"], st["s"]
                    if not st["use_psum"]:
                        st["pparts"] = None
                        continue
                    cc = st["cc"]
                    pparts = [psX.tile([128, XCH[0][1] * FW], F32, tag="px",
                                       name=f"px{cc}_{i}")
                              for i in range(2)]
                    st["pparts"] = pparts
                    pe_ops_per_half = (len(st["pe_taps"]) + st["n_fold"]
                                       + len(st["ks_taps"]))
                    mm_idx = [0, 0]
                    for hi, (r0, nr) in enumerate(XCH):
                        px = pparts[hi]
                        for i, t in enumerate(st["ks_taps"]):
                            ty, tx = divmod(t, 5)
                            s8ap = st["sf8t"][:]
                            rhs = bass.AP(
                                s8ap.tensor,
                                s8ap.offset + (r0 + ty) * SFW + tx,
                                [list(s8ap.ap[0]), [0, 2], [SFW, nr], [1, FW]])
                            nc.tensor.matmul(
                                out=px[:, :nr * FW],
                                lhsT=dr_lhsT(st["dg2s"][i][:], 0, 128),
                                rhs=rhs,
                                start=(mm_idx[hi] == 0),
                                stop=(mm_idx[hi] == pe_ops_per_half - 1),
                                perf_mode=DR)
                            mm_idx[hi] += 1
                        for i, t in enumerate(st["pe_taps"]):
                            nc.tensor.matmul(
                                out=px[:, :nr * FW],
                                lhsT=st["dgs"][i][:],
                                rhs=win_of(sf, cc, t, rows=nr, row0=r0),
                                start=(mm_idx[hi] == 0),
                                stop=(mm_idx[hi] == pe_ops_per_half - 1))
                            mm_idx[hi] += 1
                        for pr8 in st["pr8s"]:
                            rhs = bass.AP(pr8[:].tensor,
                                          pr8[:].offset + r0 * FW,
                                          [list(pr8[:].ap[0]), [625, 2],
                                           [1, nr * FW]])
                            nc.tensor.matmul(
                                out=px[:, :nr * FW],
                                lhsT=dr_lhsT(iden8x2[:], 0, 128),
                                rhs=rhs,
                                start=(mm_idx[hi] == 0),
                                stop=(mm_idx[hi] == pe_ops_per_half - 1),
                                perf_mode=DR)
                            mm_idx[hi] += 1

            def emit_xcorr_assemble(state):
                """Phase 3: ft = chain + psum partials (DVE)."""
                feat = []
                for st in state:
                    cc = st["cc"]
                    chain, pparts = st["chain"], st["pparts"]
                    ft = featp.tile([128, 625], F16, tag=f"ft{cc}",
                                    name=f"ft{cc}")
                    if pparts is not None and chain is not None:
                        for (r0, nr), px in zip(XCH, pparts):
                            srcv = bass.AP(chain.tensor, chain.offset + r0 * FW,
                                           [list(chain.ap[0]), [1, nr * FW]])
                            dv = _shifted(ft[:], r0 * FW, [[1, nr * FW]])
                            pxv = _shifted(px[:], 0, [[1, nr * FW]])
                            nc.vector.tensor_tensor(out=dv, in0=srcv, in1=pxv,
                                                    op=ALU.add)
                    elif pparts is not None:
                        for (r0, nr), px in zip(XCH, pparts):
                            dv = _shifted(ft[:], r0 * FW, [[1, nr * FW]])
                            nc.vector.tensor_copy(out=dv, in_=px[:, :nr * FW])
                    else:
                        nc.vector.tensor_copy(out=ft[:], in_=chain)
                    feat.append(ft)
                return feat

            def emit_heads(s, feat, drain=False):
                hs = []
                for co in range(2):
                    ht = hp.tile([128, 625], F16, tag=f"h{co}", name=f"h{co}")
                    for off, n in HN:
                        ps = psB.tile([128, HN[0][1]], F32, tag="hps",
                                      name="hps")
                        for ci in range(2):
                            nc.tensor.matmul(
                                out=ps[:, :n],
                                lhsT=w1_sb[ci][:, co * 128:co * 128 + 128],
                                rhs=feat[ci][:, off:off + n],
                                start=(ci == 0), stop=(ci == 1))
                        if drain and co == 1:
                            # drain: DVE is idle; relu(psum+bias) via TS
                            nc.vector.tensor_scalar(
                                out=ht[:, off:off + n], in0=ps[:, :n],
                                scalar1=bias_sb[co][:, 2:3], scalar2=0.0,
                                op0=ALU.add, op1=ALU.max)
                        else:
                            nc.scalar.activation(
                                out=ht[:, off:off + n], in_=ps[:, :n],
                                func=AF.Relu, bias=bias_sb[co][:, 2:3],
                                scale=1.0)
                    hs.append(ht)
                eng = {"gpsimd": nc.gpsimd, "scalar": nc.scalar,
                       "sync": nc.sync}[OUT_DMA_ENG]
                for co in range(2):
                    ob = obp.tile([128, 625], F32, tag=f"ob{co}", name=f"ob{co}")
                    for off, n in HN:
                        ps = psB.tile([128, HN[0][1]], F32, tag="hps",
                                      name="hps")
                        for ci in range(2):
                            nc.tensor.matmul(
                                out=ps[:, :n],
                                lhsT=w2_sb[ci][:, co * 128:co * 128 + 128],
                                rhs=hs[ci][:, off:off + n],
                                start=(ci == 0), stop=(ci == 1))
                        if drain and co == 1:
                            nc.vector.tensor_scalar(
                                out=ob[:, off:off + n], in0=ps[:, :n],
                                scalar1=bias_sb[co][:, 3:4], scalar2=None,
                                op0=ALU.add, op1=ALU.bypass)
                        else:
                            nc.scalar.activation(
                                out=ob[:, off:off + n], in_=ps[:, :n],
                                func=AF.Identity, bias=bias_sb[co][:, 3:4],
                                scale=1.0)
                        if drain:
                            eng.dma_start(
                                out=out.ap()[co, s][:, off:off + n],
                                in_=ob[:, off:off + n])
                    if not drain:
                        eng.dma_start(out=out.ap()[co, s], in_=ob[:])

            # warm the PE pstate on junk data while the weight DMAs land
            if XC_WARM > 0:
                wjunk = wp.tile([128, 512], F16, tag="wjunk", name="wjunk")
                nc.vector.memset(wjunk[:], 1.0)
                pjunk = psB.tile([128, 512], F32, tag="hps", name="pjunk")
                for _ in range(XC_WARM):
                    nc.tensor.matmul(out=pjunk[:], lhsT=wjunk[:, :128],
                                     rhs=wjunk[:], start=True, stop=True)

            prev = None
            sf0 = None
            if CK_FIRST:
                xw0 = xw0_early if xw0_early is not None else emit_conv_search_x(0)
                load_head_weights()
                emit_conv_kernel()
                sf0 = emit_conv_search(0, xw=xw0)
            else:
                sf0 = emit_conv_search(0)
                load_head_weights()
                emit_conv_kernel()
            prev_state = None
            prev_feat = None   # (s, feat) awaiting heads
            for s in range(n_samples):
                sf = sf0 if (s == 0 and sf0 is not None) else emit_conv_search(s)
                lastness = 0
                if s == n_samples - 1 and XC_LAST_MODE:
                    lastness = 2
                elif s >= n_samples - XC_TAIL:
                    lastness = 1
                state = emit_xcorr_products(s, sf, last=lastness)
                if s == 0 and XC_FILL:
                    # fill: no lag for the first sample
                    emit_xcorr_pe(state)
                    prev_feat = (0, emit_xcorr_assemble(state))
                    continue
                if prev_state is not None:
                    emit_xcorr_pe(prev_state)
                    feat = emit_xcorr_assemble(prev_state)
                    if prev_feat is not None:
                        emit_heads(prev_feat[0], prev_feat[1])
                    prev_feat = (prev_state[0]["s"], feat)
                prev_state = state
            # drain: heads(n-2) before the last sample's PE phase
            emit_heads(prev_feat[0], prev_feat[1])
            emit_xcorr_pe(prev_state)
            feat = emit_xcorr_assemble(prev_state)
            emit_heads(prev_state[0]["s"], feat, drain=True)
    _split_multi_waits(nc)
    return nc


_cache = {}


def _get_nc(n_samples=SPC):
    key = (n_samples, XC_PE16, XC_ACT8, XC_POOL8, XC_PE16_LAST, XC_TAIL,
           OUT_DMA_ENG, CK_FIRST, XC_WARM, PSA, PSB, PSX, SFB, EV_DVE,
           CS_M, XS_M, XC_KS, XC_POOL16, XC_DIAG_POOL, XC_CHAIN_POOL, XC_LAST_MODE, XC_FILL, XC_ACT_LAST,
           os.environ.get("XS0_EARLY"),
           os.environ.get("HPB"), os.environ.get("OBB"), os.environ.get("ACCB"),
           os.environ.get("FTB"), os.environ.get("XSB"),
           _HN1, _SFY0, _XCH0)
    if key not in _cache:
        _cache[key] = _build(n_samples)
    return _cache[key]


def _q8(x, scale):
    import ml_dtypes
    return (x * scale).astype(ml_dtypes.float8_e4m3)


def _prep_host(inputs):
    """Fold BN, transpose/pack weights, fp8-split conv_search operands."""
    import ml_dtypes
    f32, f16 = np.float32, np.float16
    kernel = np.asarray(inputs["kernel"], f32)
    search = np.asarray(inputs["search"], f32)

    def fold(w, g, b, m, v):
        inv = (g / np.sqrt(v + EPS)).astype(f32)
        return (w * inv[:, None, None, None]).astype(f32), (b - m * inv).astype(f32)

    wk_f, bk_f = fold(inputs["wk"], inputs["gk"], inputs["bk"], inputs["mk"], inputs["vk"])
    ws_f, bs_f = fold(inputs["ws"], inputs["gs"], inputs["bs"], inputs["ms"], inputs["vs"])
    wh1_f, bh1_f = fold(inputs["wh1"], inputs["gh"], inputs["bh"], inputs["mh"], inputs["vh"])
    wh2_f = np.asarray(inputs["wh2"], f32)[:, :, 0, 0]
    bh2_f = np.asarray(inputs["bh2"], f32)

    # fp16 lhsT packings
    wkt = np.ascontiguousarray(
        np.transpose(wk_f, (1, 2, 3, 0)).reshape(2, 128, 9 * 256)).astype(f16)
    wh1t = np.ascontiguousarray(wh1_f[:, :, 0, 0].T.reshape(2, 128, 256)).astype(f16)
    wh2t = np.ascontiguousarray(wh2_f.T.reshape(2, 128, 256)).astype(f16)

    # conv_search weights: hi/lo fp8 at shared pow2 scale, layout
    # [128ci_p, ci_chunk, tap*256 + co]
    amax_w = np.abs(ws_f).max()
    s_w = float(2.0 ** np.floor(np.log2(160.0 / max(amax_w, 1e-30))))
    wsT = np.transpose(ws_f, (1, 2, 3, 0)).reshape(2, 128, 9 * 256)  # [ci_c][ci_p][tap*256+co]
    wsT = np.ascontiguousarray(np.transpose(wsT, (1, 0, 2)))          # [128][2][2304]
    ws_hi = _q8(wsT, s_w)
    ws_lo = _q8(wsT - ws_hi.astype(f32) / s_w, s_w)
    ws_hi = ws_hi.reshape(128, 2 * 9 * 256)
    ws_lo = ws_lo.reshape(128, 2 * 9 * 256)

    # search input: pad x to 32, split hi/lo fp8 at scale S_X, layout
    # per-core [s][128ci_p][ci_chunk*992 + y*32 + x]
    spad = np.zeros((B, CIN, 31, SW), f32)
    spad[:, :, :, :31] = search
    sp = spad.reshape(B, 2, 128, 31 * SW)
    xs_hi = _q8(sp, S_X)
    xs_lo = _q8(sp - xs_hi.astype(f32) / S_X, S_X)
    # -> [B][128][2*992]
    xs_hi = np.ascontiguousarray(np.transpose(xs_hi, (0, 2, 1, 3))).reshape(B, 128, 2 * 31 * SW)
    xs_lo = np.ascontiguousarray(np.transpose(xs_lo, (0, 2, 1, 3))).reshape(B, 128, 2 * 31 * SW)

    ev_scale = np.full((256,), 1.0 / (S_X * s_w), f32)
    biases = np.ascontiguousarray(
        np.stack([bk_f, bs_f, bh1_f, bh2_f, 0.5 * bk_f, ev_scale], axis=1)
        .reshape(2, 128, 6))

    kpad = np.zeros((B, CIN, 7, KW), f16)
    kpad[:, :, :, :7] = kernel

    in_maps = []
    for core in range(N_CORES):
        sl = slice(core * SPC, (core + 1) * SPC)
        xk_c = np.ascontiguousarray(
            np.transpose(kpad[sl], (1, 0, 2, 3)).reshape(2, 128, SPC * 7 * KW))
        in_maps.append({
            "xk": xk_c,
            "xsh": np.ascontiguousarray(xs_hi[sl]),
            "xsl": np.ascontiguousarray(xs_lo[sl]),
            "wkt": wkt, "wsh": ws_hi, "wsl": ws_lo,
            "wh1t": wh1t, "wh2t": wh2t, "bias": biases,
        })
    return in_maps


def kernel(_trace=False, **inputs):
    import time as _time
    nc = _get_nc()
    in_maps = _prep_host(inputs)
    _t0 = _time.time()
    res = run_bass_kernel_spmd(nc, in_maps, core_ids=list(range(N_CORES)),
                               trace=_trace)
    kernel.last_run_s = _time.time() - _t0
    outs = []
    for core in range(N_CORES):
        o = res.results[core]["out"]  # [2, SPC, 128, 625]
        outs.append(np.transpose(o, (1, 0, 2, 3)).reshape(SPC, OC, 25, 25))
    full = np.concatenate(outs, axis=0)
    if _trace:
        kernel.last_exec_time_ns = res.exec_time_ns
        kernel.last_trace = res.instructions_and_trace
    return full


# revision 31
# speedup vs baseline: 1.0028x; 1.0028x over previous
"""Trainium2 Bass kernel for nn_DepthwiseXCorr (SiamRPN-style depthwise
cross-correlation head), data-parallel over 8 NeuronCores.

Network (per sample):
  k = relu(bn(conv3x3(kernel)))      [256,7,7]   -> [256,5,5]
  s = relu(bn(conv3x3(search)))      [256,31,31] -> [256,29,29]
  feat = depthwise_xcorr(s, k)                   -> [256,25,25]
  h = relu(bn(conv1x1(feat)))                    -> [256,25,25]
  out = conv1x1(h) + b                           -> [256,25,25]

Mapping (v2, fp8-DoubleRow design):
  - batch 128 sharded 16 samples/core across 8 cores (SPMD, no collectives)
  - BN folded into conv weights/biases on host
  - conv_search in fp8e4 DoubleRow ("cs3t"): x and w split hi+lo on host,
    three 256-deep 0.5-cyc/col passes (wh*xh + wh*xl + wl*xh) ~ fp16-exact
    at 0.75x the fp16 cycle count; conv_kernel + heads stay fp16
  - one conv tap drops its w_lo correction (CS_M=1): ~0.7e-2 extra error
    for 2 fewer DoubleRow passes per sample
  - depthwise xcorr split across four lanes per 25-tap channel-chunk:
      * XC_PE16 taps: fp16 diagonal-weight matmuls on the PE (as baseline)
      * XC_ACT8/XC_POOL8 taps: ACT/Pool copy-with-scale products written
        as fp8 into pair tiles, pairs folded into PSUM by a 65-ns
        constant-[2I,2I] fp8 DoubleRow matmul on the PE (pairs mix one ACT
        and one Pool product so folds never wait two serial ACT ops)
      * remaining taps: DVE tensor_scalar products (fp16 4x) + pair-add
        chain (fp16 2x)
    the fp8 product rounding (~3.6% rms per tap) is the dominant added
    noise; lane counts keep total rel err ~1.5e-2 vs the 2e-2 gate
  - software pipeline: PE fold phase for sample s-1 and heads for s-2 are
    emitted under conv_search(s), so the PE never head-of-line blocks on
    vector-engine product streams; out-DMAs issue from the idle SP queue
  - PSUM banks: 2 conv + 2 head + 4 xcorr partials
"""
import os
import numpy as np

import bass_rust
import concourse.bass as bass
import concourse.mybir as mybir
import concourse.tile as tile
from concourse.bass_utils import run_bass_kernel_spmd

dt = mybir.dt
F32, F16, F8 = dt.float32, dt.float16, dt.float8e4
AF = mybir.ActivationFunctionType
ALU = mybir.AluOpType
DR = mybir.MatmulPerfMode.DoubleRow

N_CORES = 8
B, CIN, HID, OC = 128, 256, 256, 256
SPC = B // N_CORES  # samples per core (16)
EPS = 1e-5

KW = 8                          # kernel input row padded 7 -> 8
SW = 32                         # search input row padded 31 -> 32
SFW = 29                        # conv_search output row
FW = 25                         # xcorr/head output row
KCOLS = SPC * 25                # conv_kernel psum free size (all samples)
S_X = 32.0                      # host fp8 scale for search input (hi and lo)

_SFY0 = int(os.environ.get("SFY0", "15"))
SFY = [(0, _SFY0), (_SFY0, 29 - _SFY0)]  # conv_search output row halves
_HN1 = int(os.environ.get("HN1", "313"))
HN = [(0, _HN1), (_HN1, 625 - _HN1)]  # head matmul N splits of 625
_XCH0 = int(os.environ.get("XCH0", "13"))
XCH = [(0, _XCH0), (_XCH0, 25 - _XCH0)]  # xcorr row halves for PE psum

# xcorr lane counts per channel-chunk (cc0, cc1); rest of 25 goes to DVE
XC_PE16 = int(os.environ.get("XC_PE16", "4"))     # fp16 diag taps / cc
XC_KS = int(os.environ.get("XC_KS", "0"))         # k-split fp8 diag taps / cc
XC_ACT8 = int(os.environ.get("XC_ACT8", "4"))     # ACT fp8-product taps / cc
XC_POOL8 = int(os.environ.get("XC_POOL8", "6"))   # Pool fp8-product taps / cc
XC_POOL16 = int(os.environ.get("XC_POOL16", "0"))  # DVE-lane products on Pool / cc
XC_DIAG_POOL = int(os.environ.get("XC_DIAG_POOL", "0"))  # build fp16 diags on Pool
XC_PE16_LAST = int(os.environ.get("XC_PE16_LAST", "4"))  # tail: extra PE taps
XC_ACT_LAST = int(os.environ.get("XC_ACT_LAST", "4"))  # tail: ACT product cap
XC_TAIL = int(os.environ.get("XC_TAIL", "1"))     # samples treated as tail
OUT_DMA_ENG = os.environ.get("OUT_DMA_ENG", "sync")  # gpsimd|scalar|sync
CK_FIRST = int(os.environ.get("CK_FIRST", "1"))   # conv_kernel before search0
XC_WARM = int(os.environ.get("XC_WARM", "8"))     # PE warm-up matmuls
PSA = int(os.environ.get("PSA", "2"))   # conv psum bufs
PSB = int(os.environ.get("PSB", "2"))   # heads psum bufs
PSX = int(os.environ.get("PSX", "4"))   # xcorr psum bufs
SFB = int(os.environ.get("SFB", "2"))   # search-feature bufs
EV_DVE = int(os.environ.get("EV_DVE", "0"))  # conv_search evacs on DVE
CS_M = int(os.environ.get("CS_M", "1"))      # taps skipping the w_lo term
XS_M = int(os.environ.get("XS_M", "0"))      # taps skipping the x_lo term
XC_CHAIN_POOL = int(os.environ.get("XC_CHAIN_POOL", "0"))  # chain TTs on Pool/cc
XC_LAST_MODE = int(os.environ.get("XC_LAST_MODE", "0"))  # 1: last sample PE+DVE only
XC_FILL = int(os.environ.get("XC_FILL", "0"))  # 1: tight (no-lag) sample 0


def _split_multi_waits(nc):
    """This walrus build accepts at most ONE sync wait per instruction;
    Tile's wait assignment can attach several. Move extras onto prepended
    same-engine NoOps (engine streams are in-order, semantics identical)."""
    n = 0
    for fn in nc.m.functions:
        for bb in fn.blocks:
            changed = False
            out = []
            for inst in bb.instructions:
                si = inst.sync_info
                waits = list(si.on_wait) if si is not None and si.on_wait else []
                if len(waits) > 1:
                    for w in waits[:-1]:
                        no = bass_rust.InstNoOp(
                            name=nc.get_next_instruction_name(), ins=[], outs=[])
                        no.engine = inst.engine
                        no.sync_info = bass_rust.SyncInfo(on_wait=[w], on_update=[])
                        out.append(no)
                    inst.sync_info = bass_rust.SyncInfo(
                        on_wait=[waits[-1]],
                        on_update=list(si.on_update) if si.on_update else [])
                    changed = True
                    n += 1
                out.append(inst)
            if changed:
                bb.instructions = out
    return n


def _shifted(ap, extra_offset, free_dims):
    """Rebuild an SBUF tile AP with a free-dim window: keep partition dim,
    replace free dims, add an element offset."""
    return bass.AP(ap.tensor, ap.offset + extra_offset,
                   [list(ap.ap[0])] + [list(d) for d in free_dims])


def _build(n_samples=SPC):
    nc = bass.Bass(trn_type="TRN2", target_bir_lowering=False, debug=False)

    xk = nc.dram_tensor("xk", [2, 128, SPC * 7 * KW], F16, kind="ExternalInput")
    # search input hi/lo fp8: [s][128][ci_chunk*992 + y*32 + x]
    xsh = nc.dram_tensor("xsh", [SPC, 128, 2 * 31 * SW], F8, kind="ExternalInput")
    xsl = nc.dram_tensor("xsl", [SPC, 128, 2 * 31 * SW], F8, kind="ExternalInput")
    wkt = nc.dram_tensor("wkt", [2, 128, 9 * 256], F16, kind="ExternalInput")
    # conv_search weights hi/lo fp8: [128ci_p][ci_chunk*2304 + tap*256 + co]
    wsh = nc.dram_tensor("wsh", [128, 2 * 9 * 256], F8, kind="ExternalInput")
    wsl = nc.dram_tensor("wsl", [128, 2 * 9 * 256], F8, kind="ExternalInput")
    wh1t = nc.dram_tensor("wh1t", [2, 128, 256], F16, kind="ExternalInput")
    wh2t = nc.dram_tensor("wh2t", [2, 128, 256], F16, kind="ExternalInput")
    # bias cols: 0=bk 1=bs 2=bh1 3=bh2 4=0.5*bk 5=conv_search evac scale
    bias = nc.dram_tensor("bias", [2, 128, 6], F32, kind="ExternalInput")
    out = nc.dram_tensor("out", [2, SPC, 128, 625], F32, kind="ExternalOutput")

    with tile.TileContext(nc) as tc:
        with tc.tile_pool(name="w", bufs=1) as wp, \
             tc.tile_pool(name="xsp", bufs=int(os.environ.get("XSB", "3"))) as xsp, \
             tc.tile_pool(name="sfp", bufs=SFB, space="SBUF") as sfp, \
             tc.tile_pool(name="prp", bufs=3) as prp, \
             tc.tile_pool(name="p8p", bufs=5) as p8p, \
             tc.tile_pool(name="accp", bufs=int(os.environ.get("ACCB", "2"))) as accp, \
             tc.tile_pool(name="featp", bufs=int(os.environ.get("FTB", "2"))) as featp, \
             tc.tile_pool(name="dgp", bufs=2) as dgp, \
             tc.tile_pool(name="sf8p", bufs=2) as sf8p, \
             tc.tile_pool(name="dg2p", bufs=2) as dg2p, \
             tc.tile_pool(name="hp", bufs=int(os.environ.get("HPB", "2"))) as hp, \
             tc.tile_pool(name="obp", bufs=int(os.environ.get("OBB", "2"))) as obp, \
             tc.tile_pool(name="psA", bufs=PSA, space="PSUM") as psA, \
             tc.tile_pool(name="psB", bufs=PSB, space="PSUM") as psB, \
             tc.tile_pool(name="psX", bufs=PSX, space="PSUM") as psX:

            # ---- resident weights / biases / kernel-branch input ----
            wk_sb, w1_sb, w2_sb, bias_sb, xk_sb = [], [], [], [], []
            for c in range(2):
                t = wp.tile([128, 9 * 256], F16, tag=f"wk{c}", name=f"wk{c}")
                nc.sync.dma_start(out=t[:], in_=wkt.ap()[c])
                wk_sb.append(t)
                t = wp.tile([128, SPC * 7 * KW], F16, tag=f"xk{c}", name=f"xk{c}")
                nc.sync.dma_start(out=t[:], in_=xk.ap()[c])
                xk_sb.append(t)
                t = wp.tile([128, 6], F32, tag=f"bias{c}", name=f"bias{c}")
                nc.sync.dma_start(out=t[:], in_=bias.ap()[c])
                bias_sb.append(t)
            XS0_EARLY = int(os.environ.get("XS0_EARLY", "0"))
            xw0_early = None
            if CK_FIRST and XS0_EARLY:
                th0 = xsp.tile([128, 2 * 31 * SW], F8, tag="xsh", name="t_xsh")
                nc.sync.dma_start(out=th0[:], in_=xsh.ap()[0])
                tl0 = xsp.tile([128, 2 * 31 * SW], F8, tag="xsl", name="t_xsl")
                nc.sync.dma_start(out=tl0[:], in_=xsl.ap()[0])
                xw0_early = (th0, tl0)
            ws_hi = wp.tile([128, 2 * 9 * 256], F8, tag="ws_hi", name="ws_hi")
            nc.sync.dma_start(out=ws_hi[:], in_=wsh.ap())
            ws_lo = wp.tile([128, 2 * 9 * 256], F8, tag="ws_lo", name="ws_lo")
            nc.sync.dma_start(out=ws_lo[:], in_=wsl.ap())

            def load_head_weights():
                for c in range(2):
                    t = wp.tile([128, 256], F16, tag=f"w1{c}", name=f"w1{c}")
                    nc.sync.dma_start(out=t[:], in_=wh1t.ap()[c])
                    w1_sb.append(t)
                    t = wp.tile([128, 256], F16, tag=f"w2{c}", name=f"w2{c}")
                    nc.sync.dma_start(out=t[:], in_=wh2t.ap()[c])
                    w2_sb.append(t)

            from concourse.masks import make_identity
            iden = wp.tile([128, 128], F32, tag="iden", name="iden")
            make_identity(nc, iden[:])
            iden16 = wp.tile([128, 128], F16, tag="iden16", name="iden16")
            nc.vector.tensor_copy(out=iden16[:], in_=iden[:])
            # constant [2I, 2I] fp8 pair-fold weights
            iden8x2 = wp.tile([128, 256], F8, tag="iden8x2", name="iden8x2")
            nc.vector.tensor_scalar_mul(iden8x2[:, :128], iden[:], 2.0)
            nc.vector.tensor_scalar_mul(iden8x2[:, 128:], iden[:], 2.0)

            def dr_lhsT(tile_ap, offset, stride):
                return bass.AP(tile_ap.tensor, tile_ap.offset + offset,
                               [list(tile_ap.ap[0]), [stride, 2], [1, 128]])

            # ---- conv_kernel: all samples batched in the free dim ----
            kf_sb = []    # fp32 k columns per cc: [128, s*25 + t]
            kf8_sb = []   # fp32 0.5*k columns per cc (fp8 product lanes)
            kl_sb = []    # fp32 k-residual columns per cc (k-split diags)

            def emit_conv_kernel():
                for co in range(2):
                    ps = psA.tile([128, KCOLS], F32, tag="ps", name="ck_ps")
                    n_mm = 0
                    for tap in range(9):
                        dy, dx = divmod(tap, 3)
                        for ci in range(2):
                            rhs = _shifted(xk_sb[ci][:], dy * KW + dx,
                                           [[7 * KW, n_samples], [KW, 5], [1, 5]])
                            lhs = wk_sb[ci][:, tap * 256 + co * 128:tap * 256 + co * 128 + 128]
                            n_cols = n_samples * 25
                            nc.tensor.matmul(out=ps[:, :n_cols], lhsT=lhs, rhs=rhs,
                                             start=(n_mm == 0), stop=(n_mm == 17))
                            n_mm += 1
                    kf = wp.tile([128, KCOLS], F32, tag=f"kf{co}", name=f"kf{co}")
                    nc.scalar.activation(out=kf[:], in_=ps[:], func=AF.Relu,
                                         bias=bias_sb[co][:, 0:1], scale=1.0)
                    kf_sb.append(kf)
                    kf8 = wp.tile([128, KCOLS], F32, tag=f"kf8{co}", name=f"kf8{co}")
                    nc.scalar.activation(out=kf8[:], in_=ps[:], func=AF.Relu,
                                         bias=bias_sb[co][:, 4:5], scale=0.5)
                    kf8_sb.append(kf8)
                    if XC_KS > 0:
                        kf8c = wp.tile([128, KCOLS], F8, tag=f"kf8c{co}",
                                       name=f"kf8c{co}")
                        nc.vector.tensor_copy(out=kf8c[:], in_=kf[:])
                        klc = wp.tile([128, KCOLS], F32, tag=f"klc{co}",
                                      name=f"klc{co}")
                        nc.vector.tensor_tensor(out=klc[:], in0=kf[:],
                                                in1=kf8c[:], op=ALU.subtract)
                        kl_sb.append(klc)

            def emit_conv_search_x(s):
                th = xsp.tile([128, 2 * 31 * SW], F8, tag="xsh", name="t_xsh")
                nc.sync.dma_start(out=th[:], in_=xsh.ap()[s])
                tl = xsp.tile([128, 2 * 31 * SW], F8, tag="xsl", name="t_xsl")
                nc.sync.dma_start(out=tl[:], in_=xsl.ap()[s])
                return (th, tl)

            def emit_conv_search(s, xw=None):
                if xw is None:
                    xw = emit_conv_search_x(s)
                th, tl = xw

                def xwin(t, ys, nr, dy, dx):
                    return bass.AP(t[:].tensor, t[:].offset + (ys + dy) * SW + dx,
                                   [list(t[:].ap[0]), [31 * SW, 2],
                                    [SW, nr], [1, SFW]])

                sf = []
                for co in range(2):
                    sft = sfp.tile([128, 29 * SFW], F16, tag=f"sf{co}",
                                   name=f"sf{co}")
                    for (ys, nr) in SFY:
                        ps = psA.tile([128, SFY[0][1] * SFW], F32, tag="ps",
                                      name="cs_ps")
                        n_tot = 27 - CS_M - XS_M
                        n_mm = 0
                        for tap in range(9):
                            dy, dx = divmod(tap, 3)
                            passes = [(ws_hi, (th, tl) if tap >= XS_M else (th,))]
                            if tap < 9 - CS_M:
                                passes.append((ws_lo, (th,)))
                            for wtile, xts in passes:
                                lhs = dr_lhsT(wtile[:], tap * 256 + co * 128,
                                              9 * 256)
                                for xt in xts:
                                    nc.tensor.matmul(
                                        out=ps[:, :nr * SFW], lhsT=lhs,
                                        rhs=xwin(xt, ys, nr, dy, dx),
                                        start=(n_mm == 0), stop=(n_mm == n_tot - 1),
                                        perf_mode=DR)
                                    n_mm += 1
                        nc.scalar.activation(
                            out=sft[:, ys * SFW:(ys + nr) * SFW],
                            in_=ps[:, :nr * SFW], func=AF.Relu,
                            bias=bias_sb[co][:, 1:2],
                            scale=bias_sb[co][:, 5:6])
                    sf.append(sft)
                return sf

            def win_of(sf, cc, t, rows=25, row0=0):
                ty, tx = divmod(t, 5)
                return _shifted(sf[cc][:], (row0 + ty) * SFW + tx,
                                [[SFW, rows], [1, FW]])

            def kcol(arr, cc, s, t):
                return arr[cc][:, s * 25 + t:s * 25 + t + 1]

            def ptile():
                t = prp.tile([128, 1250], F16, tag=f"pr{ptile.i % 6}",
                             name=f"pr{ptile.i % 6}")
                ptile.i += 1
                return t
            ptile.i = 0

            def p8tile():
                t = p8p.tile([128, 1250], F8, tag=f"p8_{p8tile.i % 6}",
                             name=f"p8_{p8tile.i % 6}")
                p8tile.i += 1
                return t
            p8tile.i = 0

            def d2tile():
                t = dg2p.tile([128, 256], F8, tag=f"d2_{d2tile.i % 10}",
                              name=f"d2_{d2tile.i % 10}")
                d2tile.i += 1
                return t
            d2tile.i = 0

            def dtile():
                t = dgp.tile([128, 128], F16, tag=f"dg{dtile.i % 12}",
                             name=f"dg{dtile.i % 12}")
                dtile.i += 1
                return t
            dtile.i = 0

            def emit_xcorr_products(s, sf, last=False):
                """Phase 1: lane assignment, fp8/fp16 products, diag builds,
                DVE chain. Returns state for the PE + assembly phases."""
                state = []
                for cc in range(2):
                    n_pe = XC_PE16
                    n_act, n_pool = XC_ACT8, XC_POOL8
                    if last >= 2:
                        # very last sample: nothing left to overlap ACT/Pool
                        # products with -- keep the drain on PE + DVE
                        n_pe = XC_PE16_LAST
                        n_act = n_pool = 0
                    elif last:
                        n_pe = XC_PE16_LAST
                        tot8 = min(n_act + n_pool, 25 - n_pe)
                        n_act = min(n_act, tot8, XC_ACT_LAST)
                        n_pool = tot8 - n_act
                    n8 = n_act + n_pool
                    if (n8 % 2) == 1:
                        n8 -= 1
                        if n_pool > 0:
                            n_pool -= 1
                        else:
                            n_act -= 1
                    n_ks = 0 if last else XC_KS
                    n_pe = min(n_pe, 25 - n8 - n_ks)
                    n_dve = 25 - n8 - n_ks - n_pe
                    dve_taps = list(range(n_dve))
                    f8_taps = list(range(n_dve, n_dve + n8))
                    ks_taps = list(range(n_dve + n8, n_dve + n8 + n_ks))
                    pe_taps = list(range(n_dve + n8 + n_ks, 25))

                    use_psum = bool(pe_taps or f8_taps or ks_taps)
                    n_fold = n8 // 2

                    # fp8 products into pair tiles; pairs mix (ACT, Pool) so a
                    # fold never waits two sequential ACT ops
                    pr8s = []
                    act_left, pool_left = n_act, n_pool
                    for j in range(n_fold):
                        ta, tb = f8_taps[2 * j], f8_taps[2 * j + 1]
                        pr8 = p8tile()
                        for slot, t in ((0, ta), (1, tb)):
                            dst = pr8[:, slot * 625:(slot + 1) * 625]
                            use_act = (act_left > 0 and (slot == 0 or pool_left == 0))
                            if use_act:
                                act_left -= 1
                                nc.scalar.activation(
                                    out=dst, in_=win_of(sf, cc, t),
                                    func=AF.Copy, scale=kcol(kf8_sb, cc, s, t))
                            else:
                                pool_left -= 1
                                nc.gpsimd.tensor_scalar(
                                    out=dst, in0=win_of(sf, cc, t),
                                    scalar1=kcol(kf8_sb, cc, s, t),
                                    scalar2=None, op0=ALU.mult, op1=ALU.bypass)
                        pr8s.append(pr8)

                    # fp16 diag builds for the PE taps (Pool has slack and
                    # the diags are consumed a lagged sample later)
                    dgs = []
                    for t in pe_taps:
                        dg = dtile()
                        if XC_DIAG_POOL:
                            nc.gpsimd.tensor_scalar(
                                out=dg[:], in0=iden16[:],
                                scalar1=kcol(kf_sb, cc, s, t), scalar2=None,
                                op0=ALU.mult, op1=ALU.bypass)
                        else:
                            nc.vector.tensor_scalar_mul(
                                dg[:], iden16[:], kcol(kf_sb, cc, s, t))
                        dgs.append(dg)

                    # k-split fp8 diag pairs + fp8 search tile (Pool-built)
                    sf8t = None
                    dg2s = []
                    if ks_taps:
                        sf8t = sf8p.tile([128, 29 * SFW], F8, tag=f"s8{cc}",
                                         name=f"s8{cc}")
                        nc.gpsimd.tensor_copy(out=sf8t[:], in_=sf[cc][:])
                        for t in ks_taps:
                            d2 = d2tile()
                            nc.gpsimd.tensor_scalar(
                                out=d2[:, :128], in0=iden16[:],
                                scalar1=kcol(kf_sb, cc, s, t), scalar2=None,
                                op0=ALU.mult, op1=ALU.bypass)
                            nc.gpsimd.tensor_scalar(
                                out=d2[:, 128:], in0=iden16[:],
                                scalar1=kcol(kl_sb, cc, s, t), scalar2=None,
                                op0=ALU.mult, op1=ALU.bypass)
                            dg2s.append(d2)

                    # DVE lane: TS products into fp16 pair tiles + TT chain;
                    # the first XC_POOL16 products go on the Pool engine
                    pairs = []
                    single = None
                    kk = 0
                    n_p16 = XC_POOL16

                    def prod16(dst, t):
                        nonlocal n_p16
                        if n_p16 > 0:
                            n_p16 -= 1
                            nc.gpsimd.tensor_scalar(
                                out=dst, in0=win_of(sf, cc, t),
                                scalar1=kcol(kf_sb, cc, s, t), scalar2=None,
                                op0=ALU.mult, op1=ALU.bypass)
                        else:
                            nc.vector.tensor_scalar_mul(
                                dst, win_of(sf, cc, t), kcol(kf_sb, cc, s, t))

                    while kk < len(dve_taps):
                        pr = ptile()
                        prod16(pr[:, :625], dve_taps[kk])
                        if kk + 1 < len(dve_taps):
                            prod16(pr[:, 625:], dve_taps[kk + 1])
                            pairs.append(pr)
                            kk += 2
                        else:
                            single = pr
                            kk += 1

                    a0 = accp.tile([128, 1250], F16, tag=f"ac{cc}a",
                                   name=f"ac{cc}a")
                    a1 = accp.tile([128, 1250], F16, tag=f"ac{cc}b",
                                   name=f"ac{cc}b")
                    accs, nxt = [a0, a1], 0
                    cur2 = None
                    n_chain_pool = XC_CHAIN_POOL
                    for pr in pairs:
                        if cur2 is None:
                            cur2 = pr[:]
                            continue
                        d = accs[nxt][:]
                        if n_chain_pool > 0:
                            n_chain_pool -= 1
                            nc.gpsimd.tensor_tensor(out=d, in0=cur2, in1=pr[:],
                                                    op=ALU.add)
                        else:
                            nc.vector.tensor_tensor(out=d, in0=cur2, in1=pr[:],
                                                    op=ALU.add)
                        cur2, nxt = d, 1 - nxt
                    # fold chain halves into [128,625]
                    chain = None
                    if cur2 is not None:
                        ch = accp.tile([128, 625], F16, tag=f"ch{cc}",
                                       name=f"ch{cc}")
                        h0 = bass.AP(cur2.tensor, cur2.offset,
                                     [list(cur2.ap[0]), [1, 625]])
                        h1 = bass.AP(cur2.tensor, cur2.offset + 625,
                                     [list(cur2.ap[0]), [1, 625]])
                        if single is not None:
                            # h0+h1 then +single via two TTs
                            nc.vector.tensor_tensor(out=ch[:], in0=h0, in1=h1,
                                                    op=ALU.add)
                            ch2 = accp.tile([128, 625], F16, tag=f"ch2{cc}",
                                            name=f"ch2{cc}")
                            nc.vector.tensor_tensor(out=ch2[:], in0=ch[:],
                                                    in1=single[:, :625],
                                                    op=ALU.add)
                            chain = ch2[:]
                        else:
                            nc.vector.tensor_tensor(out=ch[:], in0=h0, in1=h1,
                                                    op=ALU.add)
                            chain = ch[:]
                    elif single is not None:
                        chain = single[:, :625]

                    state.append(dict(cc=cc, use_psum=use_psum, chain=chain,
                                      pe_taps=pe_taps, dgs=dgs, pr8s=pr8s,
                                      ks_taps=ks_taps, dg2s=dg2s, sf8t=sf8t,
                                      n_fold=n_fold, sf=sf, s=s))
                return state

            def emit_xcorr_pe(state):
                """Phase 2: PE diag matmuls + fp8 pair folds into PSUM."""
                for st in state:
                    sf, s = st["sf"], st["s"]
                    if not st["use_psum"]:
                        st["pparts"] = None
                        continue
                    cc = st["cc"]
                    pparts = [psX.tile([128, XCH[0][1] * FW], F32, tag="px",
                                       name=f"px{cc}_{i}")
                              for i in range(2)]
                    st["pparts"] = pparts
                    pe_ops_per_half = (len(st["pe_taps"]) + st["n_fold"]
                                       + len(st["ks_taps"]))
                    mm_idx = [0, 0]
                    for hi, (r0, nr) in enumerate(XCH):
                        px = pparts[hi]
                        for i, t in enumerate(st["ks_taps"]):
                            ty, tx = divmod(t, 5)
                            s8ap = st["sf8t"][:]
                            rhs = bass.AP(
                                s8ap.tensor,
                                s8ap.offset + (r0 + ty) * SFW + tx,
                                [list(s8ap.ap[0]), [0, 2], [SFW, nr], [1, FW]])
                            nc.tensor.matmul(
                                out=px[:, :nr * FW],
                                lhsT=dr_lhsT(st["dg2s"][i][:], 0, 128),
                                rhs=rhs,
                                start=(mm_idx[hi] == 0),
                                stop=(mm_idx[hi] == pe_ops_per_half - 1),
                                perf_mode=DR)
                            mm_idx[hi] += 1
                        for i, t in enumerate(st["pe_taps"]):
                            nc.tensor.matmul(
                                out=px[:, :nr * FW],
                                lhsT=st["dgs"][i][:],
                                rhs=win_of(sf, cc, t, rows=nr, row0=r0),
                                start=(mm_idx[hi] == 0),
                                stop=(mm_idx[hi] == pe_ops_per_half - 1))
                            mm_idx[hi] += 1
                        for pr8 in st["pr8s"]:
                            rhs = bass.AP(pr8[:].tensor,
                                          pr8[:].offset + r0 * FW,
                                          [list(pr8[:].ap[0]), [625, 2],
                                           [1, nr * FW]])
                            nc.tensor.matmul(
                                out=px[:, :nr * FW],
                                lhsT=dr_lhsT(iden8x2[:], 0, 128),
                                rhs=rhs,
                                start=(mm_idx[hi] == 0),
                                stop=(mm_idx[hi] == pe_ops_per_half - 1),
                                perf_mode=DR)
                            mm_idx[hi] += 1

            def emit_xcorr_assemble(state):
                """Phase 3: ft = chain + psum partials (DVE)."""
                feat = []
                for st in state:
                    cc = st["cc"]
                    chain, pparts = st["chain"], st["pparts"]
                    ft = featp.tile([128, 625], F16, tag=f"ft{cc}",
                                    name=f"ft{cc}")
                    if pparts is not None and chain is not None:
                        for (r0, nr), px in zip(XCH, pparts):
                            srcv = bass.AP(chain.tensor, chain.offset + r0 * FW,
                                           [list(chain.ap[0]), [1, nr * FW]])
                            dv = _shifted(ft[:], r0 * FW, [[1, nr * FW]])
                            pxv = _shifted(px[:], 0, [[1, nr * FW]])
                            nc.vector.tensor_tensor(out=dv, in0=srcv, in1=pxv,
                                                    op=ALU.add)
                    elif pparts is not None:
                        for (r0, nr), px in zip(XCH, pparts):
                            dv = _shifted(ft[:], r0 * FW, [[1, nr * FW]])
                            nc.vector.tensor_copy(out=dv, in_=px[:, :nr * FW])
                    else:
                        nc.vector.tensor_copy(out=ft[:], in_=chain)
                    feat.append(ft)
                return feat

            def emit_heads(s, feat, drain=False):
                hs = []
                for co in range(2):
                    ht = hp.tile([128, 625], F16, tag=f"h{co}", name=f"h{co}")
                    for off, n in HN:
                        ps = psB.tile([128, HN[0][1]], F32, tag="hps",
                                      name="hps")
                        for ci in range(2):
                            nc.tensor.matmul(
                                out=ps[:, :n],
                                lhsT=w1_sb[ci][:, co * 128:co * 128 + 128],
                                rhs=feat[ci][:, off:off + n],
                                start=(ci == 0), stop=(ci == 1))
                        if drain and co == 1:
                            # drain: DVE is idle; relu(psum+bias) via TS
                            nc.vector.tensor_scalar(
                                out=ht[:, off:off + n], in0=ps[:, :n],
                                scalar1=bias_sb[co][:, 2:3], scalar2=0.0,
                                op0=ALU.add, op1=ALU.max)
                        else:
                            nc.scalar.activation(
                                out=ht[:, off:off + n], in_=ps[:, :n],
                                func=AF.Relu, bias=bias_sb[co][:, 2:3],
                                scale=1.0)
                    hs.append(ht)
                eng = {"gpsimd": nc.gpsimd, "scalar": nc.scalar,
                       "sync": nc.sync}[OUT_DMA_ENG]
                for co in range(2):
                    ob = obp.tile([128, 625], F32, tag=f"ob{co}", name=f"ob{co}")
                    for off, n in HN:
                        ps = psB.tile([128, HN[0][1]], F32, tag="hps",
                                      name="hps")
                        for ci in range(2):
                            nc.tensor.matmul(
                                out=ps[:, :n],
                                lhsT=w2_sb[ci][:, co * 128:co * 128 + 128],
                                rhs=hs[ci][:, off:off + n],
                                start=(ci == 0), stop=(ci == 1))
                        if drain and co == 1:
                            nc.vector.tensor_scalar(
                                out=ob[:, off:off + n], in0=ps[:, :n],
                                scalar1=bias_sb[co][:, 3:4], scalar2=None,
                                op0=ALU.add, op1=ALU.bypass)
                        else:
                            nc.scalar.activation(
                                out=ob[:, off:off + n], in_=ps[:, :n],
                                func=AF.Identity, bias=bias_sb[co][:, 3:4],
                                scale=1.0)
                        if drain:
                            eng.dma_start(
                                out=out.ap()[co, s][:, off:off + n],
                                in_=ob[:, off:off + n])
                    if not drain:
                        eng.dma_start(out=out.ap()[co, s], in_=ob[:])

            # warm the PE pstate on junk data while the weight DMAs land
            if XC_WARM > 0:
                wjunk = wp.tile([128, 512], F16, tag="wjunk", name="wjunk")
                nc.vector.memset(wjunk[:], 1.0)
                pjunk = psB.tile([128, 512], F32, tag="hps", name="pjunk")
                for _ in range(XC_WARM):
                    nc.tensor.matmul(out=pjunk[:], lhsT=wjunk[:, :128],
                                     rhs=wjunk[:], start=True, stop=True)

            prev = None
            sf0 = None
            if CK_FIRST:
                xw0 = xw0_early if xw0_early is not None else emit_conv_search_x(0)
                load_head_weights()
                emit_conv_kernel()
                sf0 = emit_conv_search(0, xw=xw0)
            else:
                sf0 = emit_conv_search(0)
                load_head_weights()
                emit_conv_kernel()
            prev_state = None
            prev_feat = None   # (s, feat) awaiting heads
            for s in range(n_samples):
                sf = sf0 if (s == 0 and sf0 is not None) else emit_conv_search(s)
                lastness = 0
                if s == n_samples - 1 and XC_LAST_MODE:
                    lastness = 2
                elif s >= n_samples - XC_TAIL:
                    lastness = 1
                state = emit_xcorr_products(s, sf, last=lastness)
                if s == 0 and XC_FILL:
                    # fill: no lag for the first sample
                    emit_xcorr_pe(state)
                    prev_feat = (0, emit_xcorr_assemble(state))
                    continue
                if prev_state is not None:
                    emit_xcorr_pe(prev_state)
                    feat = emit_xcorr_assemble(prev_state)
                    if prev_feat is not None:
                        emit_heads(prev_feat[0], prev_feat[1])
                    prev_feat = (prev_state[0]["s"], feat)
                prev_state = state
            # drain: heads(n-2) before the last sample's PE phase
            emit_heads(prev_feat[0], prev_feat[1])
            emit_xcorr_pe(prev_state)
            feat = emit_xcorr_assemble(prev_state)
            emit_heads(prev_state[0]["s"], feat, drain=True)
    _split_multi_waits(nc)
    return nc


_cache = {}


def _get_nc(n_samples=SPC):
    key = (n_samples, XC_PE16, XC_ACT8, XC_POOL8, XC_PE16_LAST, XC_TAIL,
           OUT_DMA_ENG, CK_FIRST, XC_WARM, PSA, PSB, PSX, SFB, EV_DVE,
           CS_M, XS_M, XC_KS, XC_POOL16, XC_DIAG_POOL, XC_CHAIN_POOL, XC_LAST_MODE, XC_FILL, XC_ACT_LAST,
           os.environ.get("XS0_EARLY"),
           os.environ.get("HPB"), os.environ.get("OBB"), os.environ.get("ACCB"),
           os.environ.get("FTB"), os.environ.get("XSB"),
           _HN1, _SFY0, _XCH0)
    if key not in _cache:
        _cache[key] = _build(n_samples)
    return _cache[key]


def _q8(x, scale):
    import ml_dtypes
    return (x * scale).astype(ml_dtypes.float8_e4m3)


def _prep_host(inputs):
    """Fold BN, transpose/pack weights, fp8-split conv_search operands."""
    import ml_dtypes
    f32, f16 = np.float32, np.float16
    kernel = np.asarray(inputs["kernel"], f32)
    search = np.asarray(inputs["search"], f32)

    def fold(w, g, b, m, v):
        inv = (g / np.sqrt(v + EPS)).astype(f32)
        return (w * inv[:, None, None, None]).astype(f32), (b - m * inv).astype(f32)

    wk_f, bk_f = fold(inputs["wk"], inputs["gk"], inputs["bk"], inputs["mk"], inputs["vk"])
    ws_f, bs_f = fold(inputs["ws"], inputs["gs"], inputs["bs"], inputs["ms"], inputs["vs"])
    wh1_f, bh1_f = fold(inputs["wh1"], inputs["gh"], inputs["bh"], inputs["mh"], inputs["vh"])
    wh2_f = np.asarray(inputs["wh2"], f32)[:, :, 0, 0]
    bh2_f = np.asarray(inputs["bh2"], f32)

    # fp16 lhsT packings
    wkt = np.ascontiguousarray(
        np.transpose(wk_f, (1, 2, 3, 0)).reshape(2, 128, 9 * 256)).astype(f16)
    wh1t = np.ascontiguousarray(wh1_f[:, :, 0, 0].T.reshape(2, 128, 256)).astype(f16)
    wh2t = np.ascontiguousarray(wh2_f.T.reshape(2, 128, 256)).astype(f16)

    # conv_search weights: hi/lo fp8 at shared pow2 scale, layout
    # [128ci_p, ci_chunk, tap*256 + co]
    amax_w = np.abs(ws_f).max()
    s_w = float(2.0 ** np.floor(np.log2(160.0 / max(amax_w, 1e-30))))
    wsT = np.transpose(ws_f, (1, 2, 3, 0)).reshape(2, 128, 9 * 256)  # [ci_c][ci_p][tap*256+co]
    wsT = np.ascontiguousarray(np.transpose(wsT, (1, 0, 2)))          # [128][2][2304]
    ws_hi = _q8(wsT, s_w)
    ws_lo = _q8(wsT - ws_hi.astype(f32) / s_w, s_w)
    ws_hi = ws_hi.reshape(128, 2 * 9 * 256)
    ws_lo = ws_lo.reshape(128, 2 * 9 * 256)

    # search input: pad x to 32, split hi/lo fp8 at scale S_X, layout
    # per-core [s][128ci_p][ci_chunk*992 + y*32 + x]
    spad = np.zeros((B, CIN, 31, SW), f32)
    spad[:, :, :, :31] = search
    sp = spad.reshape(B, 2, 128, 31 * SW)
    xs_hi = _q8(sp, S_X)
    xs_lo = _q8(sp - xs_hi.astype(f32) / S_X, S_X)
    # -> [B][128][2*992]
    xs_hi = np.ascontiguousarray(np.transpose(xs_hi, (0, 2, 1, 3))).reshape(B, 128, 2 * 31 * SW)
    xs_lo = np.ascontiguousarray(np.transpose(xs_lo, (0, 2, 1, 3))).reshape(B, 128, 2 * 31 * SW)

    ev_scale = np.full((256,), 1.0 / (S_X * s_w), f32)
    biases = np.ascontiguousarray(
        np.stack([bk_f, bs_f, bh1_f, bh2_f, 0.5 * bk_f, ev_scale], axis=1)
        .reshape(2, 128, 6))

    kpad = np.zeros((B, CIN, 7, KW), f16)
    kpad[:, :, :, :7] = kernel

    in_maps = []
    for core in range(N_CORES):
        sl = slice(core * SPC, (core + 1) * SPC)
        xk_c = np.ascontiguousarray(
            np.transpose(kpad[sl], (1, 0, 2, 3)).reshape(2, 128, SPC * 7 * KW))
        in_maps.append({
            "xk": xk_c,
            "xsh": np.ascontiguousarray(xs_hi[sl]),
            "xsl": np.ascontiguousarray(xs_lo[sl]),
            "wkt": wkt, "wsh": ws_hi, "wsl": ws_lo,
            "wh1t": wh1t, "wh2t": wh2t, "bias": biases,
        })
    return in_maps


def kernel(_trace=False, **inputs):
    import time as _time
    nc = _get_nc()
    in_maps = _prep_host(inputs)
    _t0 = _time.time()
    res = run_bass_kernel_spmd(nc, in_maps, core_ids=list(range(N_CORES)),
                               trace=_trace)
    kernel.last_run_s = _time.time() - _t0
    outs = []
    for core in range(N_CORES):
        o = res.results[core]["out"]  # [2, SPC, 128, 625]
        outs.append(np.transpose(o, (1, 0, 2, 3)).reshape(SPC, OC, 25, 25))
    full = np.concatenate(outs, axis=0)
    if _trace:
        kernel.last_exec_time_ns = res.exec_time_ns
        kernel.last_trace = res.instructions_and_trace
    return full


# revision 32
# speedup vs baseline: 1.0112x; 1.0084x over previous
"""Trainium2 Bass kernel for nn_DepthwiseXCorr (SiamRPN-style depthwise
cross-correlation head), data-parallel over 8 NeuronCores.

Network (per sample):
  k = relu(bn(conv3x3(kernel)))      [256,7,7]   -> [256,5,5]
  s = relu(bn(conv3x3(search)))      [256,31,31] -> [256,29,29]
  feat = depthwise_xcorr(s, k)                   -> [256,25,25]
  h = relu(bn(conv1x1(feat)))                    -> [256,25,25]
  out = conv1x1(h) + b                           -> [256,25,25]

Mapping (v2, fp8-DoubleRow design):
  - batch 128 sharded 16 samples/core across 8 cores (SPMD, no collectives)
  - BN folded into conv weights/biases on host
  - conv_search in fp8e4 DoubleRow ("cs3t"): x and w split hi+lo on host,
    three 256-deep 0.5-cyc/col passes (wh*xh + wh*xl + wl*xh) ~ fp16-exact
    at 0.75x the fp16 cycle count; conv_kernel + heads stay fp16
  - one conv tap drops its w_lo correction (CS_M=1): ~0.7e-2 extra error
    for 2 fewer DoubleRow passes per sample
  - depthwise xcorr split across four lanes per 25-tap channel-chunk:
      * XC_PE16 taps: fp16 diagonal-weight matmuls on the PE (as baseline)
      * XC_ACT8/XC_POOL8 taps: ACT/Pool copy-with-scale products written
        as fp8 into pair tiles, pairs folded into PSUM by a 65-ns
        constant-[2I,2I] fp8 DoubleRow matmul on the PE (pairs mix one ACT
        and one Pool product so folds never wait two serial ACT ops)
      * remaining taps: DVE tensor_scalar products (fp16 4x) + pair-add
        chain (fp16 2x)
    the fp8 product rounding (~3.6% rms per tap) is the dominant added
    noise; lane counts keep total rel err ~1.5e-2 vs the 2e-2 gate
  - software pipeline: PE fold phase for sample s-1 and heads for s-2 are
    emitted under conv_search(s), so the PE never head-of-line blocks on
    vector-engine product streams; out-DMAs issue from the idle SP queue
  - PSUM banks: 2 conv + 2 head + 4 xcorr partials
"""
import os
import numpy as np

import bass_rust
import concourse.bass as bass
import concourse.mybir as mybir
import concourse.tile as tile
from concourse.bass_utils import run_bass_kernel_spmd

dt = mybir.dt
F32, F16, F8 = dt.float32, dt.float16, dt.float8e4
AF = mybir.ActivationFunctionType
ALU = mybir.AluOpType
DR = mybir.MatmulPerfMode.DoubleRow

N_CORES = 8
B, CIN, HID, OC = 128, 256, 256, 256
SPC = B // N_CORES  # samples per core (16)
EPS = 1e-5

KW = 8                          # kernel input row padded 7 -> 8
SW = 32                         # search input row padded 31 -> 32
SFW = 29                        # conv_search output row
FW = 25                         # xcorr/head output row
KCOLS = SPC * 25                # conv_kernel psum free size (all samples)
S_X = 32.0                      # host fp8 scale for search input (hi and lo)

_SFY0 = int(os.environ.get("SFY0", "15"))
SFY = [(0, _SFY0), (_SFY0, 29 - _SFY0)]  # conv_search output row halves
_HN1 = int(os.environ.get("HN1", "313"))
HN = [(0, _HN1), (_HN1, 625 - _HN1)]  # head matmul N splits of 625
_XCH0 = int(os.environ.get("XCH0", "13"))
XCH = [(0, _XCH0), (_XCH0, 25 - _XCH0)]  # xcorr row halves for PE psum

# xcorr lane counts per channel-chunk (cc0, cc1); rest of 25 goes to DVE
XC_PE16 = int(os.environ.get("XC_PE16", "4"))     # fp16 diag taps / cc
XC_KS = int(os.environ.get("XC_KS", "0"))         # k-split fp8 diag taps / cc
XC_ACT8 = int(os.environ.get("XC_ACT8", "4"))     # ACT fp8-product taps / cc
XC_POOL8 = int(os.environ.get("XC_POOL8", "6"))   # Pool fp8-product taps / cc
XC_POOL16 = int(os.environ.get("XC_POOL16", "0"))  # DVE-lane products on Pool / cc
XC_DIAG_POOL = int(os.environ.get("XC_DIAG_POOL", "1"))  # build fp16 diags on Pool
XC_PE16_LAST = int(os.environ.get("XC_PE16_LAST", "4"))  # tail: extra PE taps
XC_ACT_LAST = int(os.environ.get("XC_ACT_LAST", "4"))  # tail: ACT product cap
XC_TAIL = int(os.environ.get("XC_TAIL", "1"))     # samples treated as tail
OUT_DMA_ENG = os.environ.get("OUT_DMA_ENG", "sync")  # gpsimd|scalar|sync
CK_FIRST = int(os.environ.get("CK_FIRST", "1"))   # conv_kernel before search0
XC_WARM = int(os.environ.get("XC_WARM", "8"))     # PE warm-up matmuls
PSA = int(os.environ.get("PSA", "2"))   # conv psum bufs
PSB = int(os.environ.get("PSB", "2"))   # heads psum bufs
PSX = int(os.environ.get("PSX", "4"))   # xcorr psum bufs
SFB = int(os.environ.get("SFB", "2"))   # search-feature bufs
EV_DVE = int(os.environ.get("EV_DVE", "0"))  # conv_search evacs on DVE
CS_M = int(os.environ.get("CS_M", "1"))      # taps skipping the w_lo term
XS_M = int(os.environ.get("XS_M", "0"))      # taps skipping the x_lo term
XC_CHAIN_POOL = int(os.environ.get("XC_CHAIN_POOL", "0"))  # chain TTs on Pool/cc
XC_LAST_MODE = int(os.environ.get("XC_LAST_MODE", "0"))  # 1: last sample PE+DVE only
XC_FILL = int(os.environ.get("XC_FILL", "0"))  # 1: tight (no-lag) sample 0


def _split_multi_waits(nc):
    """This walrus build accepts at most ONE sync wait per instruction;
    Tile's wait assignment can attach several. Move extras onto prepended
    same-engine NoOps (engine streams are in-order, semantics identical)."""
    n = 0
    for fn in nc.m.functions:
        for bb in fn.blocks:
            changed = False
            out = []
            for inst in bb.instructions:
                si = inst.sync_info
                waits = list(si.on_wait) if si is not None and si.on_wait else []
                if len(waits) > 1:
                    for w in waits[:-1]:
                        no = bass_rust.InstNoOp(
                            name=nc.get_next_instruction_name(), ins=[], outs=[])
                        no.engine = inst.engine
                        no.sync_info = bass_rust.SyncInfo(on_wait=[w], on_update=[])
                        out.append(no)
                    inst.sync_info = bass_rust.SyncInfo(
                        on_wait=[waits[-1]],
                        on_update=list(si.on_update) if si.on_update else [])
                    changed = True
                    n += 1
                out.append(inst)
            if changed:
                bb.instructions = out
    return n


def _shifted(ap, extra_offset, free_dims):
    """Rebuild an SBUF tile AP with a free-dim window: keep partition dim,
    replace free dims, add an element offset."""
    return bass.AP(ap.tensor, ap.offset + extra_offset,
                   [list(ap.ap[0])] + [list(d) for d in free_dims])


def _build(n_samples=SPC):
    nc = bass.Bass(trn_type="TRN2", target_bir_lowering=False, debug=False)

    xk = nc.dram_tensor("xk", [2, 128, SPC * 7 * KW], F16, kind="ExternalInput")
    # search input hi/lo fp8: [s][128][ci_chunk*992 + y*32 + x]
    xsh = nc.dram_tensor("xsh", [SPC, 128, 2 * 31 * SW], F8, kind="ExternalInput")
    xsl = nc.dram_tensor("xsl", [SPC, 128, 2 * 31 * SW], F8, kind="ExternalInput")
    wkt = nc.dram_tensor("wkt", [2, 128, 9 * 256], F16, kind="ExternalInput")
    # conv_search weights hi/lo fp8: [128ci_p][ci_chunk*2304 + tap*256 + co]
    wsh = nc.dram_tensor("wsh", [128, 2 * 9 * 256], F8, kind="ExternalInput")
    wsl = nc.dram_tensor("wsl", [128, 2 * 9 * 256], F8, kind="ExternalInput")
    wh1t = nc.dram_tensor("wh1t", [2, 128, 256], F16, kind="ExternalInput")
    wh2t = nc.dram_tensor("wh2t", [2, 128, 256], F16, kind="ExternalInput")
    # bias cols: 0=bk 1=bs 2=bh1 3=bh2 4=0.5*bk 5=conv_search evac scale
    bias = nc.dram_tensor("bias", [2, 128, 6], F32, kind="ExternalInput")
    out = nc.dram_tensor("out", [2, SPC, 128, 625], F32, kind="ExternalOutput")

    with tile.TileContext(nc) as tc:
        with tc.tile_pool(name="w", bufs=1) as wp, \
             tc.tile_pool(name="xsp", bufs=int(os.environ.get("XSB", "3"))) as xsp, \
             tc.tile_pool(name="sfp", bufs=SFB, space="SBUF") as sfp, \
             tc.tile_pool(name="prp", bufs=3) as prp, \
             tc.tile_pool(name="p8p", bufs=5) as p8p, \
             tc.tile_pool(name="accp", bufs=int(os.environ.get("ACCB", "2"))) as accp, \
             tc.tile_pool(name="featp", bufs=int(os.environ.get("FTB", "2"))) as featp, \
             tc.tile_pool(name="dgp", bufs=2) as dgp, \
             tc.tile_pool(name="sf8p", bufs=2) as sf8p, \
             tc.tile_pool(name="dg2p", bufs=2) as dg2p, \
             tc.tile_pool(name="hp", bufs=int(os.environ.get("HPB", "2"))) as hp, \
             tc.tile_pool(name="obp", bufs=int(os.environ.get("OBB", "2"))) as obp, \
             tc.tile_pool(name="psA", bufs=PSA, space="PSUM") as psA, \
             tc.tile_pool(name="psB", bufs=PSB, space="PSUM") as psB, \
             tc.tile_pool(name="psX", bufs=PSX, space="PSUM") as psX:

            # ---- resident weights / biases / kernel-branch input ----
            wk_sb, w1_sb, w2_sb, bias_sb, xk_sb = [], [], [], [], []
            for c in range(2):
                t = wp.tile([128, 9 * 256], F16, tag=f"wk{c}", name=f"wk{c}")
                nc.sync.dma_start(out=t[:], in_=wkt.ap()[c])
                wk_sb.append(t)
                t = wp.tile([128, SPC * 7 * KW], F16, tag=f"xk{c}", name=f"xk{c}")
                nc.sync.dma_start(out=t[:], in_=xk.ap()[c])
                xk_sb.append(t)
                t = wp.tile([128, 6], F32, tag=f"bias{c}", name=f"bias{c}")
                nc.sync.dma_start(out=t[:], in_=bias.ap()[c])
                bias_sb.append(t)
            XS0_EARLY = int(os.environ.get("XS0_EARLY", "0"))
            xw0_early = None
            if CK_FIRST and XS0_EARLY:
                th0 = xsp.tile([128, 2 * 31 * SW], F8, tag="xsh", name="t_xsh")
                nc.sync.dma_start(out=th0[:], in_=xsh.ap()[0])
                tl0 = xsp.tile([128, 2 * 31 * SW], F8, tag="xsl", name="t_xsl")
                nc.sync.dma_start(out=tl0[:], in_=xsl.ap()[0])
                xw0_early = (th0, tl0)
            ws_hi = wp.tile([128, 2 * 9 * 256], F8, tag="ws_hi", name="ws_hi")
            nc.sync.dma_start(out=ws_hi[:], in_=wsh.ap())
            ws_lo = wp.tile([128, 2 * 9 * 256], F8, tag="ws_lo", name="ws_lo")
            nc.sync.dma_start(out=ws_lo[:], in_=wsl.ap())

            def load_head_weights():
                for c in range(2):
                    t = wp.tile([128, 256], F16, tag=f"w1{c}", name=f"w1{c}")
                    nc.sync.dma_start(out=t[:], in_=wh1t.ap()[c])
                    w1_sb.append(t)
                    t = wp.tile([128, 256], F16, tag=f"w2{c}", name=f"w2{c}")
                    nc.sync.dma_start(out=t[:], in_=wh2t.ap()[c])
                    w2_sb.append(t)

            from concourse.masks import make_identity
            iden = wp.tile([128, 128], F32, tag="iden", name="iden")
            make_identity(nc, iden[:])
            iden16 = wp.tile([128, 128], F16, tag="iden16", name="iden16")
            nc.vector.tensor_copy(out=iden16[:], in_=iden[:])
            # constant [2I, 2I] fp8 pair-fold weights
            iden8x2 = wp.tile([128, 256], F8, tag="iden8x2", name="iden8x2")
            nc.vector.tensor_scalar_mul(iden8x2[:, :128], iden[:], 2.0)
            nc.vector.tensor_scalar_mul(iden8x2[:, 128:], iden[:], 2.0)

            def dr_lhsT(tile_ap, offset, stride):
                return bass.AP(tile_ap.tensor, tile_ap.offset + offset,
                               [list(tile_ap.ap[0]), [stride, 2], [1, 128]])

            # ---- conv_kernel: all samples batched in the free dim ----
            kf_sb = []    # fp32 k columns per cc: [128, s*25 + t]
            kf8_sb = []   # fp32 0.5*k columns per cc (fp8 product lanes)
            kl_sb = []    # fp32 k-residual columns per cc (k-split diags)

            def emit_conv_kernel():
                for co in range(2):
                    ps = psA.tile([128, KCOLS], F32, tag="ps", name="ck_ps")
                    n_mm = 0
                    for tap in range(9):
                        dy, dx = divmod(tap, 3)
                        for ci in range(2):
                            rhs = _shifted(xk_sb[ci][:], dy * KW + dx,
                                           [[7 * KW, n_samples], [KW, 5], [1, 5]])
                            lhs = wk_sb[ci][:, tap * 256 + co * 128:tap * 256 + co * 128 + 128]
                            n_cols = n_samples * 25
                            nc.tensor.matmul(out=ps[:, :n_cols], lhsT=lhs, rhs=rhs,
                                             start=(n_mm == 0), stop=(n_mm == 17))
                            n_mm += 1
                    kf = wp.tile([128, KCOLS], F32, tag=f"kf{co}", name=f"kf{co}")
                    nc.scalar.activation(out=kf[:], in_=ps[:], func=AF.Relu,
                                         bias=bias_sb[co][:, 0:1], scale=1.0)
                    kf_sb.append(kf)
                    kf8 = wp.tile([128, KCOLS], F32, tag=f"kf8{co}", name=f"kf8{co}")
                    nc.scalar.activation(out=kf8[:], in_=ps[:], func=AF.Relu,
                                         bias=bias_sb[co][:, 4:5], scale=0.5)
                    kf8_sb.append(kf8)
                    if XC_KS > 0:
                        kf8c = wp.tile([128, KCOLS], F8, tag=f"kf8c{co}",
                                       name=f"kf8c{co}")
                        nc.vector.tensor_copy(out=kf8c[:], in_=kf[:])
                        klc = wp.tile([128, KCOLS], F32, tag=f"klc{co}",
                                      name=f"klc{co}")
                        nc.vector.tensor_tensor(out=klc[:], in0=kf[:],
                                                in1=kf8c[:], op=ALU.subtract)
                        kl_sb.append(klc)

            def emit_conv_search_x(s):
                th = xsp.tile([128, 2 * 31 * SW], F8, tag="xsh", name="t_xsh")
                nc.sync.dma_start(out=th[:], in_=xsh.ap()[s])
                tl = xsp.tile([128, 2 * 31 * SW], F8, tag="xsl", name="t_xsl")
                nc.sync.dma_start(out=tl[:], in_=xsl.ap()[s])
                return (th, tl)

            def emit_conv_search(s, xw=None):
                if xw is None:
                    xw = emit_conv_search_x(s)
                th, tl = xw

                def xwin(t, ys, nr, dy, dx):
                    return bass.AP(t[:].tensor, t[:].offset + (ys + dy) * SW + dx,
                                   [list(t[:].ap[0]), [31 * SW, 2],
                                    [SW, nr], [1, SFW]])

                sf = []
                for co in range(2):
                    sft = sfp.tile([128, 29 * SFW], F16, tag=f"sf{co}",
                                   name=f"sf{co}")
                    for (ys, nr) in SFY:
                        ps = psA.tile([128, SFY[0][1] * SFW], F32, tag="ps",
                                      name="cs_ps")
                        n_tot = 27 - CS_M - XS_M
                        n_mm = 0
                        for tap in range(9):
                            dy, dx = divmod(tap, 3)
                            passes = [(ws_hi, (th, tl) if tap >= XS_M else (th,))]
                            if tap < 9 - CS_M:
                                passes.append((ws_lo, (th,)))
                            for wtile, xts in passes:
                                lhs = dr_lhsT(wtile[:], tap * 256 + co * 128,
                                              9 * 256)
                                for xt in xts:
                                    nc.tensor.matmul(
                                        out=ps[:, :nr * SFW], lhsT=lhs,
                                        rhs=xwin(xt, ys, nr, dy, dx),
                                        start=(n_mm == 0), stop=(n_mm == n_tot - 1),
                                        perf_mode=DR)
                                    n_mm += 1
                        nc.scalar.activation(
                            out=sft[:, ys * SFW:(ys + nr) * SFW],
                            in_=ps[:, :nr * SFW], func=AF.Relu,
                            bias=bias_sb[co][:, 1:2],
                            scale=bias_sb[co][:, 5:6])
                    sf.append(sft)
                return sf

            def win_of(sf, cc, t, rows=25, row0=0):
                ty, tx = divmod(t, 5)
                return _shifted(sf[cc][:], (row0 + ty) * SFW + tx,
                                [[SFW, rows], [1, FW]])

            def kcol(arr, cc, s, t):
                return arr[cc][:, s * 25 + t:s * 25 + t + 1]

            def ptile():
                t = prp.tile([128, 1250], F16, tag=f"pr{ptile.i % 6}",
                             name=f"pr{ptile.i % 6}")
                ptile.i += 1
                return t
            ptile.i = 0

            def p8tile():
                t = p8p.tile([128, 1250], F8, tag=f"p8_{p8tile.i % 6}",
                             name=f"p8_{p8tile.i % 6}")
                p8tile.i += 1
                return t
            p8tile.i = 0

            def d2tile():
                t = dg2p.tile([128, 256], F8, tag=f"d2_{d2tile.i % 10}",
                              name=f"d2_{d2tile.i % 10}")
                d2tile.i += 1
                return t
            d2tile.i = 0

            def dtile():
                t = dgp.tile([128, 128], F16, tag=f"dg{dtile.i % 12}",
                             name=f"dg{dtile.i % 12}")
                dtile.i += 1
                return t
            dtile.i = 0

            def emit_xcorr_products(s, sf, last=False):
                """Phase 1: lane assignment, fp8/fp16 products, diag builds,
                DVE chain. Returns state for the PE + assembly phases."""
                state = []
                for cc in range(2):
                    n_pe = XC_PE16
                    n_act, n_pool = XC_ACT8, XC_POOL8
                    if last >= 2:
                        # very last sample: nothing left to overlap ACT/Pool
                        # products with -- keep the drain on PE + DVE
                        n_pe = XC_PE16_LAST
                        n_act = n_pool = 0
                    elif last:
                        n_pe = XC_PE16_LAST
                        tot8 = min(n_act + n_pool, 25 - n_pe)
                        n_act = min(n_act, tot8, XC_ACT_LAST)
                        n_pool = tot8 - n_act
                    n8 = n_act + n_pool
                    if (n8 % 2) == 1:
                        n8 -= 1
                        if n_pool > 0:
                            n_pool -= 1
                        else:
                            n_act -= 1
                    n_ks = 0 if last else XC_KS
                    n_pe = min(n_pe, 25 - n8 - n_ks)
                    n_dve = 25 - n8 - n_ks - n_pe
                    dve_taps = list(range(n_dve))
                    f8_taps = list(range(n_dve, n_dve + n8))
                    ks_taps = list(range(n_dve + n8, n_dve + n8 + n_ks))
                    pe_taps = list(range(n_dve + n8 + n_ks, 25))

                    use_psum = bool(pe_taps or f8_taps or ks_taps)
                    n_fold = n8 // 2

                    # fp8 products into pair tiles; pairs mix (ACT, Pool) so a
                    # fold never waits two sequential ACT ops
                    pr8s = []
                    act_left, pool_left = n_act, n_pool
                    for j in range(n_fold):
                        ta, tb = f8_taps[2 * j], f8_taps[2 * j + 1]
                        pr8 = p8tile()
                        for slot, t in ((0, ta), (1, tb)):
                            dst = pr8[:, slot * 625:(slot + 1) * 625]
                            use_act = (act_left > 0 and (slot == 0 or pool_left == 0))
                            if use_act:
                                act_left -= 1
                                nc.scalar.activation(
                                    out=dst, in_=win_of(sf, cc, t),
                                    func=AF.Copy, scale=kcol(kf8_sb, cc, s, t))
                            else:
                                pool_left -= 1
                                nc.gpsimd.tensor_scalar(
                                    out=dst, in0=win_of(sf, cc, t),
                                    scalar1=kcol(kf8_sb, cc, s, t),
                                    scalar2=None, op0=ALU.mult, op1=ALU.bypass)
                        pr8s.append(pr8)

                    # fp16 diag builds for the PE taps (Pool has slack and
                    # the diags are consumed a lagged sample later)
                    dgs = []
                    for t in pe_taps:
                        dg = dtile()
                        if XC_DIAG_POOL:
                            nc.gpsimd.tensor_scalar(
                                out=dg[:], in0=iden16[:],
                                scalar1=kcol(kf_sb, cc, s, t), scalar2=None,
                                op0=ALU.mult, op1=ALU.bypass)
                        else:
                            nc.vector.tensor_scalar_mul(
                                dg[:], iden16[:], kcol(kf_sb, cc, s, t))
                        dgs.append(dg)

                    # k-split fp8 diag pairs + fp8 search tile (Pool-built)
                    sf8t = None
                    dg2s = []
                    if ks_taps:
                        sf8t = sf8p.tile([128, 29 * SFW], F8, tag=f"s8{cc}",
                                         name=f"s8{cc}")
                        nc.gpsimd.tensor_copy(out=sf8t[:], in_=sf[cc][:])
                        for t in ks_taps:
                            d2 = d2tile()
                            nc.gpsimd.tensor_scalar(
                                out=d2[:, :128], in0=iden16[:],
                                scalar1=kcol(kf_sb, cc, s, t), scalar2=None,
                                op0=ALU.mult, op1=ALU.bypass)
                            nc.gpsimd.tensor_scalar(
                                out=d2[:, 128:], in0=iden16[:],
                                scalar1=kcol(kl_sb, cc, s, t), scalar2=None,
                                op0=ALU.mult, op1=ALU.bypass)
                            dg2s.append(d2)

                    # DVE lane: TS products into fp16 pair tiles + TT chain;
                    # the first XC_POOL16 products go on the Pool engine
                    pairs = []
                    single = None
                    kk = 0
                    n_p16 = XC_POOL16

                    def prod16(dst, t):
                        nonlocal n_p16
                        if n_p16 > 0:
                            n_p16 -= 1
                            nc.gpsimd.tensor_scalar(
                                out=dst, in0=win_of(sf, cc, t),
                                scalar1=kcol(kf_sb, cc, s, t), scalar2=None,
                                op0=ALU.mult, op1=ALU.bypass)
                        else:
                            nc.vector.tensor_scalar_mul(
                                dst, win_of(sf, cc, t), kcol(kf_sb, cc, s, t))

                    while kk < len(dve_taps):
                        pr = ptile()
                        prod16(pr[:, :625], dve_taps[kk])
                        if kk + 1 < len(dve_taps):
                            prod16(pr[:, 625:], dve_taps[kk + 1])
                            pairs.append(pr)
                            kk += 2
                        else:
                            single = pr
                            kk += 1

                    a0 = accp.tile([128, 1250], F16, tag=f"ac{cc}a",
                                   name=f"ac{cc}a")
                    a1 = accp.tile([128, 1250], F16, tag=f"ac{cc}b",
                                   name=f"ac{cc}b")
                    accs, nxt = [a0, a1], 0
                    cur2 = None
                    n_chain_pool = XC_CHAIN_POOL
                    for pr in pairs:
                        if cur2 is None:
                            cur2 = pr[:]
                            continue
                        d = accs[nxt][:]
                        if n_chain_pool > 0:
                            n_chain_pool -= 1
                            nc.gpsimd.tensor_tensor(out=d, in0=cur2, in1=pr[:],
                                                    op=ALU.add)
                        else:
                            nc.vector.tensor_tensor(out=d, in0=cur2, in1=pr[:],
                                                    op=ALU.add)
                        cur2, nxt = d, 1 - nxt
                    # fold chain halves into [128,625]
                    chain = None
                    if cur2 is not None:
                        ch = accp.tile([128, 625], F16, tag=f"ch{cc}",
                                       name=f"ch{cc}")
                        h0 = bass.AP(cur2.tensor, cur2.offset,
                                     [list(cur2.ap[0]), [1, 625]])
                        h1 = bass.AP(cur2.tensor, cur2.offset + 625,
                                     [list(cur2.ap[0]), [1, 625]])
                        if single is not None:
                            # h0+h1 then +single via two TTs
                            nc.vector.tensor_tensor(out=ch[:], in0=h0, in1=h1,
                                                    op=ALU.add)
                            ch2 = accp.tile([128, 625], F16, tag=f"ch2{cc}",
                                            name=f"ch2{cc}")
                            nc.vector.tensor_tensor(out=ch2[:], in0=ch[:],
                                                    in1=single[:, :625],
                                                    op=ALU.add)
                            chain = ch2[:]
                        else:
                            nc.vector.tensor_tensor(out=ch[:], in0=h0, in1=h1,
                                                    op=ALU.add)
                            chain = ch[:]
                    elif single is not None:
                        chain = single[:, :625]

                    state.append(dict(cc=cc, use_psum=use_psum, chain=chain,
                                      pe_taps=pe_taps, dgs=dgs, pr8s=pr8s,
                                      ks_taps=ks_taps, dg2s=dg2s, sf8t=sf8t,
                                      n_fold=n_fold, sf=sf, s=s))
                return state

            def emit_xcorr_pe(state):
                """Phase 2: PE diag matmuls + fp8 pair folds into PSUM."""
                for st in state:
                    sf, s = st["sf"], st["s"]
                    if not st["use_psum"]:
                        st["pparts"] = None
                        continue
                    cc = st["cc"]
                    pparts = [psX.tile([128, XCH[0][1] * FW], F32, tag="px",
                                       name=f"px{cc}_{i}")
                              for i in range(2)]
                    st["pparts"] = pparts
                    pe_ops_per_half = (len(st["pe_taps"]) + st["n_fold"]
                                       + len(st["ks_taps"]))
                    mm_idx = [0, 0]
                    for hi, (r0, nr) in enumerate(XCH):
                        px = pparts[hi]
                        for i, t in enumerate(st["ks_taps"]):
                            ty, tx = divmod(t, 5)
                            s8ap = st["sf8t"][:]
                            rhs = bass.AP(
                                s8ap.tensor,
                                s8ap.offset + (r0 + ty) * SFW + tx,
                                [list(s8ap.ap[0]), [0, 2], [SFW, nr], [1, FW]])
                            nc.tensor.matmul(
                                out=px[:, :nr * FW],
                                lhsT=dr_lhsT(st["dg2s"][i][:], 0, 128),
                                rhs=rhs,
                                start=(mm_idx[hi] == 0),
                                stop=(mm_idx[hi] == pe_ops_per_half - 1),
                                perf_mode=DR)
                            mm_idx[hi] += 1
                        for i, t in enumerate(st["pe_taps"]):
                            nc.tensor.matmul(
                                out=px[:, :nr * FW],
                                lhsT=st["dgs"][i][:],
                                rhs=win_of(sf, cc, t, rows=nr, row0=r0),
                                start=(mm_idx[hi] == 0),
                                stop=(mm_idx[hi] == pe_ops_per_half - 1))
                            mm_idx[hi] += 1
                        for pr8 in st["pr8s"]:
                            rhs = bass.AP(pr8[:].tensor,
                                          pr8[:].offset + r0 * FW,
                                          [list(pr8[:].ap[0]), [625, 2],
                                           [1, nr * FW]])
                            nc.tensor.matmul(
                                out=px[:, :nr * FW],
                                lhsT=dr_lhsT(iden8x2[:], 0, 128),
                                rhs=rhs,
                                start=(mm_idx[hi] == 0),
                                stop=(mm_idx[hi] == pe_ops_per_half - 1),
                                perf_mode=DR)
                            mm_idx[hi] += 1

            def emit_xcorr_assemble(state):
                """Phase 3: ft = chain + psum partials (DVE)."""
                feat = []
                for st in state:
                    cc = st["cc"]
                    chain, pparts = st["chain"], st["pparts"]
                    ft = featp.tile([128, 625], F16, tag=f"ft{cc}",
                                    name=f"ft{cc}")
                    if pparts is not None and chain is not None:
                        for (r0, nr), px in zip(XCH, pparts):
                            srcv = bass.AP(chain.tensor, chain.offset + r0 * FW,
                                           [list(chain.ap[0]), [1, nr * FW]])
                            dv = _shifted(ft[:], r0 * FW, [[1, nr * FW]])
                            pxv = _shifted(px[:], 0, [[1, nr * FW]])
                            nc.vector.tensor_tensor(out=dv, in0=srcv, in1=pxv,
                                                    op=ALU.add)
                    elif pparts is not None:
                        for (r0, nr), px in zip(XCH, pparts):
                            dv = _shifted(ft[:], r0 * FW, [[1, nr * FW]])
                            nc.vector.tensor_copy(out=dv, in_=px[:, :nr * FW])
                    else:
                        nc.vector.tensor_copy(out=ft[:], in_=chain)
                    feat.append(ft)
                return feat

            def emit_heads(s, feat, drain=False):
                hs = []
                for co in range(2):
                    ht = hp.tile([128, 625], F16, tag=f"h{co}", name=f"h{co}")
                    for off, n in HN:
                        ps = psB.tile([128, HN[0][1]], F32, tag="hps",
                                      name="hps")
                        for ci in range(2):
                            nc.tensor.matmul(
                                out=ps[:, :n],
                                lhsT=w1_sb[ci][:, co * 128:co * 128 + 128],
                                rhs=feat[ci][:, off:off + n],
                                start=(ci == 0), stop=(ci == 1))
                        if drain and co == 1:
                            # drain: DVE is idle; relu(psum+bias) via TS
                            nc.vector.tensor_scalar(
                                out=ht[:, off:off + n], in0=ps[:, :n],
                                scalar1=bias_sb[co][:, 2:3], scalar2=0.0,
                                op0=ALU.add, op1=ALU.max)
                        else:
                            nc.scalar.activation(
                                out=ht[:, off:off + n], in_=ps[:, :n],
                                func=AF.Relu, bias=bias_sb[co][:, 2:3],
                                scale=1.0)
                    hs.append(ht)
                eng = {"gpsimd": nc.gpsimd, "scalar": nc.scalar,
                       "sync": nc.sync}[OUT_DMA_ENG]
                for co in range(2):
                    ob = obp.tile([128, 625], F32, tag=f"ob{co}", name=f"ob{co}")
                    for off, n in HN:
                        ps = psB.tile([128, HN[0][1]], F32, tag="hps",
                                      name="hps")
                        for ci in range(2):
                            nc.tensor.matmul(
                                out=ps[:, :n],
                                lhsT=w2_sb[ci][:, co * 128:co * 128 + 128],
                                rhs=hs[ci][:, off:off + n],
                                start=(ci == 0), stop=(ci == 1))
                        if drain and co == 1:
                            nc.vector.tensor_scalar(
                                out=ob[:, off:off + n], in0=ps[:, :n],
                                scalar1=bias_sb[co][:, 3:4], scalar2=None,
                                op0=ALU.add, op1=ALU.bypass)
                        else:
                            nc.scalar.activation(
                                out=ob[:, off:off + n], in_=ps[:, :n],
                                func=AF.Identity, bias=bias_sb[co][:, 3:4],
                                scale=1.0)
                        if drain:
                            eng.dma_start(
                                out=out.ap()[co, s][:, off:off + n],
                                in_=ob[:, off:off + n])
                    if not drain:
                        eng.dma_start(out=out.ap()[co, s], in_=ob[:])

            # warm the PE pstate on junk data while the weight DMAs land
            if XC_WARM > 0:
                wjunk = wp.tile([128, 512], F16, tag="wjunk", name="wjunk")
                nc.vector.memset(wjunk[:], 1.0)
                pjunk = psB.tile([128, 512], F32, tag="hps", name="pjunk")
                for _ in range(XC_WARM):
                    nc.tensor.matmul(out=pjunk[:], lhsT=wjunk[:, :128],
                                     rhs=wjunk[:], start=True, stop=True)

            prev = None
            sf0 = None
            if CK_FIRST:
                xw0 = xw0_early if xw0_early is not None else emit_conv_search_x(0)
                load_head_weights()
                emit_conv_kernel()
                sf0 = emit_conv_search(0, xw=xw0)
            else:
                sf0 = emit_conv_search(0)
                load_head_weights()
                emit_conv_kernel()
            prev_state = None
            prev_feat = None   # (s, feat) awaiting heads
            for s in range(n_samples):
                sf = sf0 if (s == 0 and sf0 is not None) else emit_conv_search(s)
                lastness = 0
                if s == n_samples - 1 and XC_LAST_MODE:
                    lastness = 2
                elif s >= n_samples - XC_TAIL:
                    lastness = 1
                state = emit_xcorr_products(s, sf, last=lastness)
                if s == 0 and XC_FILL:
                    # fill: no lag for the first sample
                    emit_xcorr_pe(state)
                    prev_feat = (0, emit_xcorr_assemble(state))
                    continue
                if prev_state is not None:
                    emit_xcorr_pe(prev_state)
                    feat = emit_xcorr_assemble(prev_state)
                    if prev_feat is not None:
                        emit_heads(prev_feat[0], prev_feat[1])
                    prev_feat = (prev_state[0]["s"], feat)
                prev_state = state
            # drain: heads(n-2) before the last sample's PE phase
            emit_heads(prev_feat[0], prev_feat[1])
            emit_xcorr_pe(prev_state)
            feat = emit_xcorr_assemble(prev_state)
            emit_heads(prev_state[0]["s"], feat, drain=True)
    _split_multi_waits(nc)
    return nc


_cache = {}


def _get_nc(n_samples=SPC):
    key = (n_samples, XC_PE16, XC_ACT8, XC_POOL8, XC_PE16_LAST, XC_TAIL,
           OUT_DMA_ENG, CK_FIRST, XC_WARM, PSA, PSB, PSX, SFB, EV_DVE,
           CS_M, XS_M, XC_KS, XC_POOL16, XC_DIAG_POOL, XC_CHAIN_POOL, XC_LAST_MODE, XC_FILL, XC_ACT_LAST,
           os.environ.get("XS0_EARLY"),
           os.environ.get("HPB"), os.environ.get("OBB"), os.environ.get("ACCB"),
           os.environ.get("FTB"), os.environ.get("XSB"),
           _HN1, _SFY0, _XCH0)
    if key not in _cache:
        _cache[key] = _build(n_samples)
    return _cache[key]


def _q8(x, scale):
    import ml_dtypes
    return (x * scale).astype(ml_dtypes.float8_e4m3)


def _prep_host(inputs):
    """Fold BN, transpose/pack weights, fp8-split conv_search operands."""
    import ml_dtypes
    f32, f16 = np.float32, np.float16
    kernel = np.asarray(inputs["kernel"], f32)
    search = np.asarray(inputs["search"], f32)

    def fold(w, g, b, m, v):
        inv = (g / np.sqrt(v + EPS)).astype(f32)
        return (w * inv[:, None, None, None]).astype(f32), (b - m * inv).astype(f32)

    wk_f, bk_f = fold(inputs["wk"], inputs["gk"], inputs["bk"], inputs["mk"], inputs["vk"])
    ws_f, bs_f = fold(inputs["ws"], inputs["gs"], inputs["bs"], inputs["ms"], inputs["vs"])
    wh1_f, bh1_f = fold(inputs["wh1"], inputs["gh"], inputs["bh"], inputs["mh"], inputs["vh"])
    wh2_f = np.asarray(inputs["wh2"], f32)[:, :, 0, 0]
    bh2_f = np.asarray(inputs["bh2"], f32)

    # fp16 lhsT packings
    wkt = np.ascontiguousarray(
        np.transpose(wk_f, (1, 2, 3, 0)).reshape(2, 128, 9 * 256)).astype(f16)
    wh1t = np.ascontiguousarray(wh1_f[:, :, 0, 0].T.reshape(2, 128, 256)).astype(f16)
    wh2t = np.ascontiguousarray(wh2_f.T.reshape(2, 128, 256)).astype(f16)

    # conv_search weights: hi/lo fp8 at shared pow2 scale, layout
    # [128ci_p, ci_chunk, tap*256 + co]
    amax_w = np.abs(ws_f).max()
    s_w = float(2.0 ** np.floor(np.log2(160.0 / max(amax_w, 1e-30))))
    wsT = np.transpose(ws_f, (1, 2, 3, 0)).reshape(2, 128, 9 * 256)  # [ci_c][ci_p][tap*256+co]
    wsT = np.ascontiguousarray(np.transpose(wsT, (1, 0, 2)))          # [128][2][2304]
    ws_hi = _q8(wsT, s_w)
    ws_lo = _q8(wsT - ws_hi.astype(f32) / s_w, s_w)
    ws_hi = ws_hi.reshape(128, 2 * 9 * 256)
    ws_lo = ws_lo.reshape(128, 2 * 9 * 256)

    # search input: pad x to 32, split hi/lo fp8 at scale S_X, layout
    # per-core [s][128ci_p][ci_chunk*992 + y*32 + x]
    spad = np.zeros((B, CIN, 31, SW), f32)
    spad[:, :, :, :31] = search
    sp = spad.reshape(B, 2, 128, 31 * SW)
    xs_hi = _q8(sp, S_X)
    xs_lo = _q8(sp - xs_hi.astype(f32) / S_X, S_X)
    # -> [B][128][2*992]
    xs_hi = np.ascontiguousarray(np.transpose(xs_hi, (0, 2, 1, 3))).reshape(B, 128, 2 * 31 * SW)
    xs_lo = np.ascontiguousarray(np.transpose(xs_lo, (0, 2, 1, 3))).reshape(B, 128, 2 * 31 * SW)

    ev_scale = np.full((256,), 1.0 / (S_X * s_w), f32)
    biases = np.ascontiguousarray(
        np.stack([bk_f, bs_f, bh1_f, bh2_f, 0.5 * bk_f, ev_scale], axis=1)
        .reshape(2, 128, 6))

    kpad = np.zeros((B, CIN, 7, KW), f16)
    kpad[:, :, :, :7] = kernel

    in_maps = []
    for core in range(N_CORES):
        sl = slice(core * SPC, (core + 1) * SPC)
        xk_c = np.ascontiguousarray(
            np.transpose(kpad[sl], (1, 0, 2, 3)).reshape(2, 128, SPC * 7 * KW))
        in_maps.append({
            "xk": xk_c,
            "xsh": np.ascontiguousarray(xs_hi[sl]),
            "xsl": np.ascontiguousarray(xs_lo[sl]),
            "wkt": wkt, "wsh": ws_hi, "wsl": ws_lo,
            "wh1t": wh1t, "wh2t": wh2t, "bias": biases,
        })
    return in_maps


def kernel(_trace=False, **inputs):
    import time as _time
    nc = _get_nc()
    in_maps = _prep_host(inputs)
    _t0 = _time.time()
    res = run_bass_kernel_spmd(nc, in_maps, core_ids=list(range(N_CORES)),
                               trace=_trace)
    kernel.last_run_s = _time.time() - _t0
    outs = []
    for core in range(N_CORES):
        o = res.results[core]["out"]  # [2, SPC, 128, 625]
        outs.append(np.transpose(o, (1, 0, 2, 3)).reshape(SPC, OC, 25, 25))
    full = np.concatenate(outs, axis=0)
    if _trace:
        kernel.last_exec_time_ns = res.exec_time_ns
        kernel.last_trace = res.instructions_and_trace
    return full
